# revision 26
# baseline (speedup 1.0000x reference)
"""Trainium2 Bass kernel for MIGAttention (topk token masking + GQA attention).

Shapes (hardcoded): B=4, N=2048, C=1024, H=16 heads, HKV=4 kv-heads, DH=64,
keep-ratio 0.7 -> k = 1433 selected tokens per batch row.

Sharding: 8 cores = (batch b in 0..3) x (query-half h in 0..1).  Each core
receives x[b].T with token columns rolled by h*1024 so that its own query
half always occupies columns 0..1023 -> a single SPMD program for all cores.

v4: topk selects 1433 of 2048 tokens; masked tokens have zero K/V rows.  The
kernel compacts selected tokens on device (parallel [128,16] prefix-sum with
a PE-transpose cross-partition step + local_scatter) and runs attention over
NSEL=1536 gathered key slots (12 chunks).  Junk tail slots have zero K/V so
their logits are exactly 0; the remaining 512 masked keys are a static +512
on the denominator.  DRAM round-trips are ordered with explicit DMA
completion semaphores (DMA->DMA deps through DRAM are otherwise racy).

Q/K logits run in bf16 (fp8 DoubleRow gives no speedup when the second
k-tile is zero: DR streams 2x data at 2x rate).  ALL att@V matmuls run fp8
DoubleRow over chunk PAIRS (256 slots contracted per stream = 2x PE):
every chunk produces the fp8 residual r = (p-1)/SV; even chunk-pairs compute
p exactly on ACT (exp) then subtract on DVE, odd pairs use a 2nd-order expm1
polynomial entirely on DVE.  att@V accumulates r x (SV*v) plus the exact
bf16 rank-1 correction Sum(v) (ones column carries the denominator).  This
keeps fp8 error on the small residual r instead of p and balances
Scalar/DVE/PE.
"""

import contextlib
import sys

import ml_dtypes
import numpy as np

if "/opt/trn_rl_repo" not in sys.path:
    sys.path.insert(0, "/opt/trn_rl_repo")

import concourse.bass as bass  # noqa: F401
import concourse.bass_isa as bass_isa
import concourse.mybir as mybir
from concourse import bacc
from concourse.tile import TileContext

F32 = mybir.dt.float32
F32R = mybir.dt.float32r
BF16 = mybir.dt.bfloat16
F8E4 = mybir.dt.float8e4
I32 = mybir.dt.int32
I16 = mybir.dt.int16
AF = mybir.ActivationFunctionType
ALU = mybir.AluOpType
DR = mybir.MatmulPerfMode.DoubleRow

B, N, C = 4, 2048, 1024
H, HKV, DH = 16, 4, 64
NQ = N // 2          # queries per core
KSEL = 1433          # max(1, int(N * 0.7))
CC = C // 128        # contraction chunks (8)
KC = N // 128        # dense token chunks (16)
KCG = 12             # gathered key chunks (1536 slots >= KSEL)
NSEL = KCG * 128     # 1536
QT_D = H * DH        # 1024
KV_D = HKV * DH      # 256
N_ROUNDS = 4         # topk threshold rounds (8/128^4 ~ 3e-8 << min gap 3.5e-6)
LO0, W0 = -4.0, 8.0  # initial logit search interval

SV = 4.0             # fp8 residual scale: r8 = (p-1)/SV, v8f = v*SV
EXP_SCALE = 1.0 / np.sqrt(DH)
GSUB, NSUB = 384, NSEL // 384  # SWDGE sub-gather split


def _emit(nc, tc, ctx, io):
    xT, wq, wk, wv, rw, wo, out_d = (
        io["xT"], io["wq"], io["wk"], io["wv"], io["rw"], io["wo"], io["out"])

    # ---------------- long-lived pools ----------------
    const = ctx.enter_context(tc.tile_pool(name="const", bufs=1))
    small = ctx.enter_context(tc.tile_pool(name="small", bufs=1))
    big = ctx.enter_context(tc.tile_pool(name="big", bufs=1))
    dram = ctx.enter_context(tc.tile_pool(name="dram", bufs=1, space="DRAM"))

    px_ctx = contextlib.ExitStack()   # xT/xb (alive through projections)
    pm_ctx = contextlib.ExitStack()   # masks/compaction scratch
    pa_ctx = contextlib.ExitStack()   # router/refinement scratch
    pg_ctx = contextlib.ExitStack()   # proj staging + psum
    pw_ctx = contextlib.ExitStack()   # wq/wk/wv
    px = px_ctx.enter_context(tc.tile_pool(name="px", bufs=1))
    pm = pm_ctx.enter_context(tc.tile_pool(name="pm", bufs=1))
    pw = pw_ctx.enter_context(tc.tile_pool(name="pw", bufs=1))
    pa = pa_ctx.enter_context(tc.tile_pool(name="pa", bufs=1))
    psum_r = pa_ctx.enter_context(tc.tile_pool(name="psum_r", bufs=1, space="PSUM"))

    # ---------------- constants ----------------
    ones_row = const.tile([1, 128], F32)
    nc.vector.memset(ones_row, 1.0)
    ones512 = const.tile([1, 512], BF16)
    nc.vector.memset(ones512, 1.0)
    onescol_bf = const.tile([128, 1], BF16)
    nc.vector.memset(onescol_bf, 1.0)
    iota128_i = const.tile([128, 1], I32)
    nc.gpsimd.iota(iota128_i, pattern=[[0, 1]], base=1, channel_multiplier=1)
    iota128 = const.tile([128, 1], F32)
    nc.vector.tensor_copy(iota128, iota128_i)
    # token id in p-major [128,16] layout: t = 16*p + i
    tpm_i = const.tile([128, 16], I32)
    nc.gpsimd.iota(tpm_i, pattern=[[1, 16]], base=0, channel_multiplier=16)
    tpm_f = const.tile([128, 16], F32)
    nc.vector.tensor_copy(tpm_f, tpm_i)
    sel8 = const.tile([16, CC, 128], F32R)
    nc.sync.dma_start(sel8, io["sel8"].bitcast(F32R))
    eye128 = const.tile([128, 128], F32)
    nc.sync.dma_start(eye128, io["eye"])
    # scatter source data: token ids 0..N-1 (row 0 used)
    sc_data = const.tile([16, N], I16)
    nc.gpsimd.iota(sc_data, pattern=[[1, N]], base=0, channel_multiplier=0)

    # ---------------- DRAM scratch ----------------
    m_dram = dram.tile([N], F32)
    pos_dram = dram.tile([N], I16)
    idx16_dram = dram.tile([NSEL], I16)
    k_dram = dram.tile([N, KV_D], BF16)
    v_dram = dram.tile([N, KV_D], BF16)

    # ---------------- router: logits = x @ rw, exact fp32 ----------------
    rw_sb = pa.tile([128, CC], F32)
    for cc in range(CC):
        sl = slice(cc * 128, (cc + 1) * 128)
        nc.sync.dma_start(rw_sb[:, cc:cc + 1], rw[sl, :])
    # Single fp32 x load feeds both the exact-fp32 PE router matmuls (the
    # instruction structure must match the reference summation closely: batch
    # 0's topk threshold gap is 3.5e-6, so any other reduction order flips a
    # near-tie) and, via engine casts, the bf16 working copy for projections.
    logits_sb = pa.tile([1, N], F32)
    wk_sb = pw.tile([128, CC, KV_D], BF16)
    wv_sb = pw.tile([128, CC, KV_D], BF16)
    wq_sb = pw.tile([128, CC, QT_D], BF16)
    xr_ctx = contextlib.ExitStack()
    xr_pool = xr_ctx.enter_context(tc.tile_pool(name="xr_pool", bufs=4))
    xb = px.tile([128, CC, N], BF16)
    rps = [psum_r.tile([1, 512], F32, tag=f"router_ps{g}", name=f"router_ps{g}")
           for g in range(4)]
    for cc in range(CC):
        xr = xr_pool.tile([128, N], F32, tag="xr", name=f"xr{cc}")
        eng = nc.sync if cc % 2 == 0 else nc.scalar
        eng.dma_start(xr, xT[cc * 128:(cc + 1) * 128, :])
        # prefetch projection weights between x chunks (V first: its
        # projection is the first PE consumer after the router)
        sl = slice(cc * 128, (cc + 1) * 128)
        nc.sync.dma_start(wv_sb[:, cc, :], wv[sl, :])
        nc.sync.dma_start(wk_sb[:, cc, :], wk[sl, :])
        nc.sync.dma_start(wq_sb[:, cc, :], wq[sl, :])
        for g in range(4):
            nc.tensor.matmul(
                rps[g], rw_sb[:, cc:cc + 1], xr[:, g * 512:(g + 1) * 512],
                start=(cc == 0), stop=(cc == CC - 1))
        if cc % 2 == 0:
            nc.scalar.copy(xb[:, cc, :], xr)
        else:
            nc.vector.tensor_copy(xb[:, cc, :], xr)
    for g in range(4):
        nc.vector.tensor_copy(logits_sb[:, g * 512:(g + 1) * 512], rps[g])
    xr_ctx.close()

    # replicate logits across all 128 partitions (K=1 matmul broadcast)
    lrep = pa.tile([128, N], F32)
    for g in range(4):
        ps = psum_r.tile([128, 512], F32, tag="bcast_ps")
        nc.tensor.matmul(ps, ones_row, logits_sb[:, g * 512:(g + 1) * 512],
                         start=True, stop=True)
        nc.vector.tensor_copy(lrep[:, g * 512:(g + 1) * 512], ps)

    # ---------------- topk threshold refinement ----------------
    # invariant: v* (the KSEL-th largest logit) is in (lo, lo + w]
    lo = small.tile([128, 1], F32)
    nc.vector.memset(lo, LO0)
    neg_edges = small.tile([128, 1], F32)
    acc = small.tile([128, 1], F32)
    sel = small.tile([128, 1], F32)
    ssum = small.tile([128, 1], F32)
    sign_scr = pa.tile([128, N], BF16)  # Sign output is never read
    thr_acc = float(2 * KSEL - N)  # acc = #gt - #lt ; acc>=thr <=> #gt>=KSEL
    for r in range(N_ROUNDS):
        wstep = W0 / (128.0 ** (r + 1))
        nc.vector.scalar_tensor_tensor(
            neg_edges, iota128, -wstep, lo, op0=ALU.mult, op1=ALU.subtract)
        nc.scalar.activation(sign_scr, lrep, AF.Sign, bias=neg_edges,
                             scale=1.0, accum_out=acc)
        nc.vector.tensor_single_scalar(sel, acc, thr_acc, op=ALU.is_ge)
        nc.gpsimd.partition_all_reduce(ssum, sel, channels=128,
                                       reduce_op=bass_isa.ReduceOp.add)
        nc.vector.scalar_tensor_tensor(
            lo, ssum, wstep, lo, op0=ALU.mult, op1=ALU.add)

    # m = (logit > lo) * sigmoid(logit)   per token, replicated on partitions
    grep = pa.tile([128, N], F32)
    nc.scalar.activation(grep, lrep, AF.Sigmoid)
    m_rep = pm.tile([128, N], F32)
    nc.vector.scalar_tensor_tensor(
        m_rep, lrep, lo, grep, op0=ALU.is_gt, op1=ALU.mult)

    # token-major copies (sem-ordered DRAM bounce): m_v[p,i] = m[i*128+p]
    # for the K/V evict scales; lpm[p,i] = m[16p+i] for the compaction.
    nc.scalar.dma_start(m_dram, m_rep[0:1, :])
    pc = pm
    m_v = pc.tile([128, KC], F32)
    nc.scalar.dma_start(m_v, m_dram.rearrange("(i p) -> p i", p=128))
    lpm = pc.tile([128, 16], F32)
    nc.scalar.dma_start(lpm, m_dram.rearrange("(p i) -> p i", i=16))
    pa_ctx.close()  # frees router scratch + its PSUM banks

    # ---------------- compaction part A (DVE): per-partition scan ---------
    m01 = pc.tile([128, 16], F32)
    nc.vector.tensor_scalar(m01, lpm, 0.0, None, op0=ALU.is_gt)
    csum = pc.tile([128, 16], F32)
    nc.vector.tensor_tensor_scan(csum, m01, m01, 0.0,
                                 op0=ALU.add, op1=ALU.bypass)
    tot128 = pc.tile([128, 1], F32)
    nc.gpsimd.partition_all_reduce(tot128, csum[:, 15:16], channels=128,
                                   reduce_op=bass_isa.ReduceOp.add)

    stg_pool = pg_ctx.enter_context(tc.tile_pool(name="stg_pool", bufs=3))
    psum1 = pg_ctx.enter_context(tc.tile_pool(name="psum1", bufs=5, space="PSUM"))
    psum_p = pg_ctx.enter_context(tc.tile_pool(name="psum_p", bufs=1, space="PSUM"))
    # ---------------- V projection, token-major -> DRAM rows --------------
    for i2 in range(KC // 2):
        vps = psum1.tile([128, 512], F32, tag="proj_ps", name=f"v_ps{i2}")
        for half in range(2):
            i = 2 * i2 + half
            ts_ = slice(i * 128, (i + 1) * 128)
            vp = vps[:, half * KV_D:(half + 1) * KV_D]
            for cc in range(CC):
                nc.tensor.matmul(vp, xb[:, cc, ts_], wv_sb[:, cc, :],
                                 start=(cc == 0), stop=(cc == CC - 1))
        vstage = stg_pool.tile([128, 2, KV_D], BF16, tag="vstage",
                               name=f"vstage{i2}")
        nc.scalar.activation(vstage[:, 0, :], vps[:, 0:KV_D], AF.Identity,
                             scale=m_v[:, 2 * i2:2 * i2 + 1])
        nc.scalar.activation(vstage[:, 1, :], vps[:, KV_D:2 * KV_D],
                             AF.Identity, scale=m_v[:, 2 * i2 + 1:2 * i2 + 2])
        nc.sync.dma_start(
            v_dram[2 * i2 * 128:(2 * i2 + 2) * 128, :]
            .rearrange("(c t) d -> t c d", t=128), vstage)

    # ------- compaction part B: cross-partition prefix via PE matmuls -----
    # exclusive prefix over per-partition totals: col -> row (matmul with
    # identity rhs), row scan (DVE), subtract, row -> col (K=1 matmul).
    # PE-queue position (after V proj) keeps these topk-dependent matmuls
    # from stalling the projection matmuls.
    row_ps = psum_p.tile([1, 128], F32, tag="prefix", name="row_ps")
    nc.tensor.matmul(row_ps, csum[:, 15:16], eye128, start=True, stop=True)
    incl_row = pc.tile([1, 128], F32)
    nc.vector.tensor_copy(incl_row, row_ps)
    incl2 = pc.tile([1, 128], F32)
    nc.vector.tensor_tensor_scan(incl2, incl_row, incl_row, 0.0,
                                 op0=ALU.add, op1=ALU.bypass)
    excl_row = pc.tile([1, 128], F32)
    nc.vector.tensor_tensor(excl_row, incl2, incl_row, op=ALU.subtract)
    col_ps = psum_p.tile([128, 1], F32, tag="prefix2", name="col_ps")
    nc.tensor.matmul(col_ps, excl_row, ones_row[0:1, 0:1],
                     start=True, stop=True)
    excl_col = pc.tile([128, 1], F32)
    nc.vector.tensor_copy(excl_col, col_ps)
    # pos[t] = selected ? csel-1 : total + t - csel, clamped to < NSEL
    csel = pc.tile([128, 16], F32)
    nc.vector.tensor_scalar(csel, csum, excl_col, None, op0=ALU.add)
    pos = pc.tile([128, 16], F32)
    nc.vector.tensor_tensor(pos, tpm_f, csel, op=ALU.subtract)
    nc.vector.tensor_scalar(pos, pos, tot128, None, op0=ALU.add)
    scr = pc.tile([128, 16], F32)
    nc.vector.tensor_scalar(scr, csel, -1.0, None, op0=ALU.add)  # csel-1
    mi32 = pc.tile([128, 16], I32)
    nc.vector.tensor_copy(mi32, m01)
    nc.vector.copy_predicated(pos, mi32, scr)
    nc.vector.tensor_single_scalar(scr, pos, float(NSEL), op=ALU.is_lt)
    nc.vector.scalar_tensor_tensor(pos, pos, 1.0, scr,
                                   op0=ALU.add, op1=ALU.mult)
    pos16 = pc.tile([128, 16], I16)
    nc.vector.tensor_scalar(pos16, pos, -1.0, None, op0=ALU.add)
    nc.scalar.dma_start(pos_dram.rearrange("(p i) -> p i", i=16), pos16)
    sc_idx = pc.tile([16, N], I16)
    nc.vector.memset(sc_idx, -1)
    nc.scalar.dma_start(sc_idx[0:1, :], pos_dram.rearrange("n -> () n"))

    # ---------------- K projection, token-major -> DRAM rows --------------
    # evicts on DVE (tensor_scalar with per-partition m) to keep the ACT
    # queue free for the V evicts running in the same window.
    for i2 in range(KC // 2):
        kps = psum1.tile([128, 512], F32, tag="proj_ps", name=f"k_ps{i2}")
        for half in range(2):
            i = 2 * i2 + half
            ts_ = slice(i * 128, (i + 1) * 128)
            kp = kps[:, half * KV_D:(half + 1) * KV_D]
            for cc in range(CC):
                nc.tensor.matmul(kp, xb[:, cc, ts_], wk_sb[:, cc, :],
                                 start=(cc == 0), stop=(cc == CC - 1))
        kstage = stg_pool.tile([128, 2, KV_D], BF16, tag="kstage",
                               name=f"kstage{i2}")
        nc.vector.tensor_scalar(kstage[:, 0, :], kps[:, 0:KV_D],
                                m_v[:, 2 * i2:2 * i2 + 1], None, op0=ALU.mult)
        nc.vector.tensor_scalar(kstage[:, 1, :], kps[:, KV_D:2 * KV_D],
                                m_v[:, 2 * i2 + 1:2 * i2 + 2], None,
                                op0=ALU.mult)
        nc.sync.dma_start(
            k_dram[2 * i2 * 128:(2 * i2 + 2) * 128, :]
            .rearrange("(c t) d -> t c d", t=128), kstage)

    # ---------------- scatter -> gather index list ----------------
    sc_out = pc.tile([16, NSEL], I16)
    nc.gpsimd.local_scatter(sc_out, sc_data, sc_idx, channels=16,
                            num_elems=NSEL, num_idxs=N)
    nc.scalar.dma_start(idx16_dram, sc_out[0:1, :])
    # wrapped gather-index layout: idx j -> [j % 16, j // 16], replicated
    idx16_sb = big.tile([128, NSEL // 16], I16)
    for k in range(8):
        nc.scalar.dma_start(idx16_sb[16 * k:16 * (k + 1), :],
                            idx16_dram.rearrange("(s p) -> p s", p=16))

    # ---------------- QT projection ----------------
    # Slot layout is permuted so each q-head lands on the same partition range
    # as its GQA kv-head in KT: head h -> slot (h%4)+4*(h//8), partition base
    # ((h//4)%2)*64.  Slot j holds heads (ha, ha+4), ha = j if j<4 else j+4.
    qt_sb = big.tile([128, H // 2, NQ], BF16)
    for j in range(H // 2):
        for g in range(NQ // 512):
            ps = psum1.tile([128, 512], F32, tag="proj_ps",
                            name=f"q_ps{j}_{g}")
            qs = slice(g * 512, (g + 1) * 512)
            for cc in range(CC):
                nc.tensor.matmul(
                    ps, wq_sb[:, cc, j * 128:(j + 1) * 128],
                    xb[:, cc, qs],
                    start=(cc == 0), stop=(cc == CC - 1))
            nc.vector.tensor_tensor(qt_sb[:, j, qs], ps, m_rep[:, qs],
                                    op=ALU.mult)

    # ---------------- K gathers (SWDGE) ----------------
    # transpose mode: out[d % 128, d // 128, slot] = k[idx[slot], d] -> the
    # [dims-on-partitions, token-free] layout the logits matmuls need.
    # kt: two parity copies with the other 64-partition half zeroed, so the
    # logits matmul contracts a full K=128 (partner q rows hit zeros).
    kt = big.tile([128, 2, 2, NSEL], BF16)    # [p, par, j, t]
    nc.vector.memset(kt[64:128, 0, :, :].bitcast(I32), 0)
    nc.vector.memset(kt[0:64, 1, :, :].bitcast(I32), 0)
    for g in range(NSUB):
        ic = idx16_sb[:, g * (GSUB // 16):(g + 1) * (GSUB // 16)]
        gs = slice(g * GSUB, (g + 1) * GSUB)
        kt_g = stg_pool.tile([128, 2, GSUB], BF16, tag="kt_g",
                             name=f"kt_g{g}")
        nc.gpsimd.dma_gather(kt_g, k_dram[:, :], ic,
                             GSUB, GSUB, elem_size=KV_D, transpose=True)
        nc.vector.tensor_copy(kt[0:64, 0, :, gs], kt_g[0:64, :, :])
        nc.vector.tensor_copy(kt[64:128, 1, :, gs], kt_g[64:128, :, :])

    # ---------------- V gathers (SWDGE) ----------------
    v8_sb = big.tile([128, KCG, HKV, 128], BF16)  # [slot, chunk, hk, d|ones]
    nc.vector.memset(v8_sb.bitcast(I32), 0)
    nc.vector.memset(v8_sb[:, :, :, 64:65], 1.0)
    v8g = stg_pool.tile([128, KCG, KV_D], BF16, tag="v8g")
    for g in range(NSUB):
        ic = idx16_sb[:, g * (GSUB // 16):(g + 1) * (GSUB // 16)]
        cs = slice(g * (GSUB // 128), (g + 1) * (GSUB // 128))
        nc.gpsimd.dma_gather(v8g[:, cs, :], v_dram[:, :], ic,
                             GSUB, GSUB, elem_size=KV_D, transpose=False)
        nc.vector.tensor_copy(
            v8_sb[:, cs, :, 0:64],
            v8g[:, cs, :].rearrange("p c (h e) -> p c h e", e=64))

    # fp8 V copies (x SV) + exact Sum(v) over all chunks (rank-1 correction)
    v8f = big.tile([128, KCG // 2, 2, HKV, 128], F8E4)
    vsum_sb = big.tile([1, HKV, 128], BF16)
    vs_ps = psum_p.tile([1, 512], F32, tag="vs", name="vs_ps")
    for cp in range(KCG // 2):
        for s in range(2):
            rc = 2 * cp + s
            nc.vector.tensor_scalar(
                v8f[:, cp, s, :, :], v8_sb[:, rc, :, :], SV, None,
                op0=ALU.mult)
            nc.tensor.matmul(vs_ps, onescol_bf, v8_sb[:, rc, :, :],
                             start=(cp == 0 and s == 0),
                             stop=(cp == KCG // 2 - 1 and s == 1))
    nc.vector.tensor_copy(vsum_sb, vs_ps)
    pg_ctx.close()
    pw_ctx.close()
    pm_ctx.close()
    px_ctx.close()  # free xT + phase-1 PSUM

    # ---------------- phase 2: attention ----------------
    ph2_ctx = contextlib.ExitStack()
    ph2 = ph2_ctx.enter_context(tc.tile_pool(name="ph2", bufs=1))
    wo_sb = ph2.tile([128, CC, C], F32R)
    for cc in range(CC):
        nc.sync.dma_start(wo_sb[:, cc, :],
                          wo[cc * 128:(cc + 1) * 128, :].bitcast(F32R))

    patt_ctx = contextlib.ExitStack()
    scr_pool = patt_ctx.enter_context(tc.tile_pool(name="scr_pool", bufs=4))
    p_pool = patt_ctx.enter_context(tc.tile_pool(name="p_pool", bufs=3))
    lg_pool = patt_ctx.enter_context(
        tc.tile_pool(name="lg_pool", bufs=2, space="PSUM"))
    att_pool = patt_ctx.enter_context(
        tc.tile_pool(name="att_pool", bufs=1, space="PSUM"))
    oT_sb = ph2.tile([128, CC, NQ], F32R)
    denom_sb = ph2.tile([16, NQ], F32)

    pair_heads = [(ha, ha + 4) for ha in (0, 1, 2, 3, 8, 9, 10, 11)]
    unit_ctr = 0  # global unit counter: 2 of every 3 units take the exact route
    for hp, pair in enumerate(pair_heads):
        j = pair[0] // 8
        att_ps = [att_pool.tile([128, NQ], F32, tag=f"att{m}",
                                name=f"att{hp}_{m}")
                  for m in range(2)]
        pend = []  # pipelined attv matmuls: emitted one chunk-pair behind
        for cp in range(KCG // 2):
            r8 = p_pool.tile([128, 2, 2, NQ], F8E4, tag="r8",
                             name=f"r8_{hp}_{cp}")
            for s in range(2):
                kc = 2 * cp + s
                lg = [lg_pool.tile([128, NQ], F32, tag="lg",
                                   name=f"lg{hp}_{kc}_{m2}")
                      for m2 in range(2)]
                for m in range(2):
                    jq = (pair[m] % 4) + 4 * (pair[m] // 8)
                    for g in range(NQ // 512):
                        qs = slice(g * 512, (g + 1) * 512)
                        nc.tensor.matmul(
                            lg[m][:, qs],
                            kt[:, m, j, kc * 128:(kc + 1) * 128],
                            qt_sb[:, jq, qs],
                            start=True, stop=True)
                for m in range(2):
                    if unit_ctr % 3 != 2:
                        # exact route: p = exp on ACT (frees the lg PSUM
                        # after one op), r8 = (p-1)/SV on DVE from SBUF
                        p_sb = scr_pool.tile([128, NQ], BF16, tag="p_sb",
                                             name=f"p_{hp}_{kc}_{m}")
                        nc.scalar.activation(p_sb, lg[m], AF.Exp,
                                             scale=EXP_SCALE)
                        nc.gpsimd.tensor_scalar(
                            r8[:, s, m, :], p_sb, 1.0 / SV, -1.0 / SV,
                            op0=ALU.mult, op1=ALU.add)
                    else:
                        # poly route on DVE: r = z*(1 + z/2), z = y*scale
                        racc = scr_pool.tile([128, 1], F32, tag="racc",
                                             name=f"racc{hp}{kc}")
                        tq = scr_pool.tile([128, NQ], BF16, tag="tq",
                                           name=f"tq{hp}_{kc}_{m}")
                        nc.vector.tensor_scalar(
                            tq, lg[m], float(EXP_SCALE / 2.0), 1.0,
                            op0=ALU.mult, op1=ALU.add)
                        nc.vector.affine_mul_reduce(
                            r8[:, s, m, :], racc, lg[m], tq,
                            scale=float(EXP_SCALE / SV), bias=0.0)
                    unit_ctr += 1

            def attv(cp=cp, r8=r8):
                for m in range(2):
                    hk = pair[m] // 4
                    for g in range(NQ // 512):
                        qs = slice(g * 512, (g + 1) * 512)
                        nc.tensor.matmul(
                            att_ps[m][:, qs],
                            v8f[:, cp, :, hk, :],
                            r8[:, :, m, qs],
                            start=(cp == 0), stop=False, perf_mode=DR,
                            skip_group_check=True)

            pend.append(attv)
            while len(pend) > 2:
                pend.pop(0)()
        for f in pend:
            f()
        # rank-1 correction: + Sum_{chunks} v (incl. slot count in the ones
        # column) to every query column
        for m in range(2):
            hk = pair[m] // 4
            for g in range(NQ // 512):
                nc.tensor.matmul(
                    att_ps[m][:, g * 512:(g + 1) * 512],
                    vsum_sb[0:1, hk, :], ones512,
                    start=False, stop=True, skip_group_check=True)
        # fast evict: copy [65, NQ] psum -> sbuf scratch, stash denom row,
        # numerator into oT unscaled; 1/denom once after all pairs.
        for m in range(2):
            h = pair[m]
            scr65 = scr_pool.tile([65, NQ], F32R, tag="scr65",
                                  name=f"scr65_{hp}_{m}")
            nc.vector.tensor_copy(scr65, att_ps[m][0:65, :])
            nc.sync.dma_start(denom_sb[h:h + 1, :],
                              scr65[64:65, :].bitcast(F32))
            if h % 2 == 0:
                nc.vector.tensor_copy(oT_sb[0:64, h // 2, :], scr65[0:64, :])
            else:
                # partition shift 0 -> 64 must go through DMA
                nc.sync.dma_start(oT_sb[64:128, h // 2, :], scr65[0:64, :])

    # denominator: + (N - NSEL) for the never-gathered masked keys, then
    # one batched reciprocal and a per-d-chunk broadcast multiply.
    nc.vector.tensor_scalar(denom_sb, denom_sb, float(N - NSEL), None,
                            op0=ALU.add)
    rec16 = ph2.tile([16, NQ], F32R)
    rec16_f = ph2.tile([16, NQ], F32)
    with nc.allow_low_precision(reason="2e-5 rel err << output tolerance"):
        nc.vector.reciprocal_approx_fast(out=rec16_f, in_=denom_sb)
    nc.vector.tensor_copy(rec16, rec16_f)
    for dd in range(CC):
        for g in range(NQ // 512):
            bps = lg_pool.tile([128, 512], F32, tag="lg", name=f"bps{dd}_{g}")
            nc.tensor.matmul(
                bps, sel8[:, dd, :], rec16[:, g * 512:(g + 1) * 512],
                start=True, stop=True)
            sl = slice(g * 512, (g + 1) * 512)
            nc.vector.tensor_tensor(
                oT_sb[:, dd, sl], oT_sb[:, dd, sl], bps, op=ALU.mult)
    patt_ctx.close()
    # ---------------- phase 3: output projection ----------------
    ph3_ctx = contextlib.ExitStack()
    psum3 = ph3_ctx.enter_context(tc.tile_pool(name="psum3", bufs=4, space="PSUM"))
    out_pool = ph3_ctx.enter_context(tc.tile_pool(name="out_pool", bufs=2))
    for tt in range(NQ // 128):
        out_sb = out_pool.tile([128, C], F32, tag="out_sb", name=f"out_sb{tt}")
        for og in range(C // 512):
            ps = psum3.tile([128, 512], F32, tag="out_ps", name=f"out_ps{tt}_{og}")
            for dd in range(CC):
                nc.tensor.matmul(
                    ps, oT_sb[:, dd, tt * 128:(tt + 1) * 128],
                    wo_sb[:, dd, og * 512:(og + 1) * 512],
                    start=(dd == 0), stop=(dd == CC - 1))
            nc.scalar.copy(out_sb[:, og * 512:(og + 1) * 512], ps)
        nc.sync.dma_start(out_d[tt * 128:(tt + 1) * 128, :], out_sb)
    ph3_ctx.close()
    ph2_ctx.close()


_NC = None


def build_program():
    global _NC
    if _NC is not None:
        return _NC
    from contextlib import ExitStack

    nc = bacc.Bacc("TRN2", target_bir_lowering=False, debug=False,
                   num_devices=8)
    io = {
        "xT": nc.dram_tensor("xT", (C, N), F32, kind="ExternalInput").ap(),
        "wq": nc.dram_tensor("wq", (C, QT_D), BF16, kind="ExternalInput").ap(),
        "wk": nc.dram_tensor("wk", (C, KV_D), BF16, kind="ExternalInput").ap(),
        "wv": nc.dram_tensor("wv", (C, KV_D), BF16, kind="ExternalInput").ap(),
        "rw": nc.dram_tensor("rw", (C, 1), F32, kind="ExternalInput").ap(),
        "wo": nc.dram_tensor("wo", (C, C), F32, kind="ExternalInput").ap(),
        "sel8": nc.dram_tensor("sel8", (16, CC, 128), F32,
                               kind="ExternalInput").ap(),
        "eye": nc.dram_tensor("eye", (128, 128), F32,
                              kind="ExternalInput").ap(),
        "out": nc.dram_tensor("out", (NQ, C), F32, kind="ExternalOutput").ap(),
    }
    with TileContext(nc) as tc:
        with ExitStack() as ctx:
            _emit(nc, tc, ctx, io)
    nc.compile()
    _NC = nc
    return nc


def _permute_wq(wq):
    """Column-permute wq so QT slot j's 128 cols = heads (ha, ha+4) contig."""
    wq = np.asarray(wq, np.float32).reshape(C, H, DH)
    order = []
    for j in range(H // 2):
        ha = j if j < 4 else j + 4
        order += [ha, ha + 4]
    return np.ascontiguousarray(wq[:, order, :].reshape(C, H * DH))


def make_in_maps(x, router_w, wq, wk, wv, wo):
    wq = _permute_wq(wq)
    in_maps = []
    for core in range(8):
        b, h = core // 2, core % 2
        xT_core = np.ascontiguousarray(
            np.roll(np.asarray(x[b], np.float32).T, -h * NQ, axis=1))
        sel8 = np.zeros((16, CC, 128), np.float32)
        for dd in range(CC):
            for p in range(128):
                sel8[2 * dd + p // 64, dd, p] = 1.0
        in_maps.append({
            "xT": xT_core,
            "sel8": sel8,
            "eye": np.eye(128, dtype=np.float32),
            "wq": np.ascontiguousarray(np.asarray(wq, np.float32),
                                       dtype=ml_dtypes.bfloat16),
            "wk": np.ascontiguousarray(np.asarray(wk, np.float32),
                                       dtype=ml_dtypes.bfloat16),
            "wv": np.ascontiguousarray(np.asarray(wv, np.float32),
                                       dtype=ml_dtypes.bfloat16),
            "rw": np.ascontiguousarray(router_w, dtype=np.float32),
            "wo": np.ascontiguousarray(wo, dtype=np.float32),
        })
    return in_maps


def _numpy_fallback(x, router_w, router_b, wq, bq, wk, bk, wv, bv, wo, bo):
    x = np.asarray(x, np.float32)
    gate = 1.0 / (1.0 + np.exp(-(x @ router_w + router_b)))
    xg = x * gate
    scores = gate[..., 0]
    idx = np.argsort(-scores, axis=-1, kind="stable")[:, :KSEL]
    mask = np.zeros((x.shape[0], x.shape[1]), np.float32)
    np.put_along_axis(mask, idx, 1.0, axis=1)
    xg = xg * mask[..., None]
    q = (xg @ wq + bq).reshape(B, N, H, DH)
    kk = np.repeat((xg @ wk + bk).reshape(B, N, HKV, DH), H // HKV, axis=2)
    v = np.repeat((xg @ wv + bv).reshape(B, N, HKV, DH), H // HKV, axis=2)
    att = np.einsum("bqhd,bkhd->bhqk", q, kk) / np.float32(np.sqrt(DH))
    att = att - att.max(-1, keepdims=True)
    att = np.exp(att)
    att = att / att.sum(-1, keepdims=True)
    o = np.einsum("bhqk,bkhd->bqhd", att, v).reshape(B, N, C)
    return (o @ wo + bo).astype(np.float32)


def kernel(x, router_w, router_b, wq, bq, wk, bk, wv, bv, wo, bo):
    x = np.asarray(x)
    biases = [router_b, bq, bk, bv, bo]
    if any(float(np.abs(np.asarray(t)).max()) != 0.0 for t in biases):
        # The device program folds away the (identically zero) biases; fall
        # back to an exact host implementation if that assumption breaks.
        return _numpy_fallback(x, router_w, router_b, wq, bq, wk, bk, wv, bv,
                               wo, bo)

    from concourse import bass_utils

    nc = build_program()
    in_maps = make_in_maps(x, router_w, wq, wk, wv, wo)
    res = bass_utils.run_bass_kernel_spmd(nc, in_maps, core_ids=list(range(8)))
    out = np.empty((B, N, C), np.float32)
    for core in range(8):
        b, h = core // 2, core % 2
        out[b, h * NQ:(h + 1) * NQ, :] = res.results[core]["out"]
    return out


# revision 27
# speedup vs baseline: 1.1334x; 1.1334x over previous
"""Trainium2 Bass kernel for MIGAttention (topk token masking + GQA attention).

Shapes (hardcoded): B=4, N=2048, C=1024, H=16 heads, HKV=4 kv-heads, DH=64,
keep-ratio 0.7 -> k = 1433 selected tokens per batch row.

Sharding: 8 cores = (batch b in 0..3) x (query-half h in 0..1).  Each core
receives x[b].T with token columns rolled by h*1024 so that its own query
half always occupies columns 0..1023 -> a single SPMD program for all cores.

Key structure (v2): the top-k mask selects 1433 of 2048 tokens; masked tokens
have zero K/V rows.  The kernel compacts the selected tokens on device
(prefix-sum + indirect-DMA scatter/gather) and runs attention over only
NSEL=1536 gathered key slots (12 chunks instead of 16).  Junk tail slots
(gathered masked tokens) have zero K/V so their logits are exactly 0 and
p=exp(0)=1, standing in one-for-one for masked tokens in the softmax
denominator; the remaining 2048-1536=512 masked keys are a static +512
constant added to the denominator.

QK_FP8: Q/K are quantized to fp8e4 at the projection evict and the logits
matmuls run in DoubleRow perf mode (2x).  R_PAIRS: the last 2*R_PAIRS key
chunks compute r = expm1(y) ~ y + y^2/2 on the Vector engine (one fused
affine_mul op straight from PSUM) in fp8, and their att@V uses fp8 DoubleRow
with the exact Sum(v) rank-1 correction folded into the same PSUM
accumulation.  This keeps all fp8 error on the small residual r (rms ~0.1)
instead of p (~1.0).
"""

import contextlib
import sys

import ml_dtypes
import numpy as np

if "/opt/trn_rl_repo" not in sys.path:
    sys.path.insert(0, "/opt/trn_rl_repo")

import concourse.bass as bass  # noqa: F401
import concourse.bass_isa as bass_isa
import concourse.mybir as mybir
from concourse import bacc
from concourse.tile import TileContext

F32 = mybir.dt.float32
F32R = mybir.dt.float32r
BF16 = mybir.dt.bfloat16
F8E4 = mybir.dt.float8e4
I32 = mybir.dt.int32
I16 = mybir.dt.int16
AF = mybir.ActivationFunctionType
ALU = mybir.AluOpType
DR = mybir.MatmulPerfMode.DoubleRow

B, N, C = 4, 2048, 1024
H, HKV, DH = 16, 4, 64
NQ = N // 2          # queries per core
KSEL = 1433          # max(1, int(N * 0.7))
CC = C // 128        # contraction chunks (8)
KC = N // 128        # dense token chunks (16)
KCG = 12             # gathered key chunks (1536 slots >= KSEL)
NSEL = KCG * 128     # 1536
QT_D = H * DH        # 1024
KV_D = HKV * DH      # 256
N_ROUNDS = 4         # topk threshold refinement rounds (8/128^4 ~ 3e-8 << min topk gap 3.5e-6)
LO0, W0 = -4.0, 8.0  # initial logit search interval (logit std ~0.65)

# ---- tuning flags ----
QK_FP8 = True        # fp8 Q/K + DoubleRow logits matmuls
R_PAIRS = 0          # of the 12 chunks, last 2*R_PAIRS use the DVE r-route
P_KC = KCG - 2 * R_PAIRS
SQK = 32.0           # fp8 quantization scale for q/k
SV = 4.0             # fp8 v scale for the r-route (r is divided by SV)
EXP_SCALE = 1.0 / (np.sqrt(DH) * (SQK * SQK if QK_FP8 else 1.0))
KDT = F8E4 if QK_FP8 else BF16
GSUB, NSUB = 384, NSEL // 384  # SWDGE sub-gather split


def _emit(nc, tc, ctx, io):
    xT, wq, wk, wv, rw, wo, out_d = (
        io["xT"], io["wq"], io["wk"], io["wv"], io["rw"], io["wo"], io["out"])

    # ---------------- long-lived pools ----------------
    const = ctx.enter_context(tc.tile_pool(name="const", bufs=1))
    small = ctx.enter_context(tc.tile_pool(name="small", bufs=1))
    big = ctx.enter_context(tc.tile_pool(name="big", bufs=1))
    dram = ctx.enter_context(tc.tile_pool(name="dram", bufs=1, space="DRAM"))

    # nesting (LIFO): px > pm > {pa, pcs, pg > {pq, pkv}}
    px_ctx = contextlib.ExitStack()   # xT (alive through projections+gather)
    pm_ctx = contextlib.ExitStack()   # m_rep/m_v/compaction sbuf scratch
    pa_ctx = contextlib.ExitStack()   # router/refinement scratch
    pg_ctx = contextlib.ExitStack()   # proj staging + transpose psum
    pq_ctx = contextlib.ExitStack()   # wq
    pkv_ctx = contextlib.ExitStack()  # wk, wv
    px = px_ctx.enter_context(tc.tile_pool(name="px", bufs=1))
    pm = pm_ctx.enter_context(tc.tile_pool(name="pm", bufs=1))
    pa = pa_ctx.enter_context(tc.tile_pool(name="pa", bufs=1))
    psum_r = pa_ctx.enter_context(tc.tile_pool(name="psum_r", bufs=1, space="PSUM"))

    # ---------------- constants ----------------
    ones_row = const.tile([1, 128], F32)
    nc.vector.memset(ones_row, 1.0)
    ones512 = const.tile([1, 512], BF16)
    nc.vector.memset(ones512, 1.0)
    onescol_bf = const.tile([128, 1], BF16)
    nc.vector.memset(onescol_bf, 1.0)
    iota128_i = const.tile([128, 1], I32)
    nc.gpsimd.iota(iota128_i, pattern=[[0, 1]], base=1, channel_multiplier=1)
    iota128 = const.tile([128, 1], F32)
    nc.vector.tensor_copy(iota128, iota128_i)
    sel8 = const.tile([16, CC, 128], F32R)
    nc.sync.dma_start(sel8, io["sel8"].bitcast(F32R))

    # ---------------- DRAM scratch ----------------
    m_dram = dram.tile([N], F32)
    idx16_dram = dram.tile([NSEL], I16)
    k_dram = dram.tile([N, KV_D], BF16)
    v_dram = dram.tile([N, KV_D], BF16)

    # ---------------- router: logits = x @ rw, exact fp32 ----------------
    rw_sb = pa.tile([128, CC], F32)
    for cc in range(CC):
        sl = slice(cc * 128, (cc + 1) * 128)
        nc.sync.dma_start(rw_sb[:, cc:cc + 1], rw[sl, :])
    # Single fp32 x load feeds both the exact-fp32 PE router matmuls (the
    # instruction structure must match the reference summation closely: batch
    # 0's topk threshold gap is 3.5e-6, so any other reduction order flips a
    # near-tie - measured both for f32r and for a DVE-partials variant) and,
    # via engine casts, the bf16 working copy the projections use.
    logits_sb = pa.tile([1, N], F32)
    xr_pool = pa_ctx.enter_context(tc.tile_pool(name="xr_pool", bufs=3))
    xb = px.tile([128, CC, N], BF16)
    rps = [psum_r.tile([1, 512], F32, tag=f"router_ps{g}", name=f"router_ps{g}")
           for g in range(4)]
    for cc in range(CC):
        xr = xr_pool.tile([128, N], F32, tag="xr", name=f"xr{cc}")
        eng = nc.sync if cc % 2 == 0 else nc.scalar
        eng.dma_start(xr, xT[cc * 128:(cc + 1) * 128, :])
        for g in range(4):
            nc.tensor.matmul(
                rps[g], rw_sb[:, cc:cc + 1], xr[:, g * 512:(g + 1) * 512],
                start=(cc == 0), stop=(cc == CC - 1))
        if cc % 2 == 0:
            nc.scalar.copy(xb[:, cc, :], xr)
        else:
            nc.vector.tensor_copy(xb[:, cc, :], xr)
    for g in range(4):
        nc.vector.tensor_copy(logits_sb[:, g * 512:(g + 1) * 512], rps[g])

    # replicate logits across all 128 partitions (K=1 matmul broadcast)
    lrep = pa.tile([128, N], F32)
    for g in range(4):
        ps = psum_r.tile([128, 512], F32, tag="bcast_ps")
        nc.tensor.matmul(ps, ones_row, logits_sb[:, g * 512:(g + 1) * 512],
                         start=True, stop=True)
        nc.vector.tensor_copy(lrep[:, g * 512:(g + 1) * 512], ps)

    # ---------------- topk threshold refinement ----------------
    # invariant: v* (the KSEL-th largest logit) is in (lo, lo + w]
    lo = small.tile([128, 1], F32)
    nc.vector.memset(lo, LO0)
    neg_edges = small.tile([128, 1], F32)
    acc = small.tile([128, 1], F32)
    sel = small.tile([128, 1], F32)
    ssum = small.tile([128, 1], F32)
    sign_scr = pa.tile([128, N], BF16)  # Sign output is never read
    thr_acc = float(2 * KSEL - N)  # acc = #gt - #lt ; acc>=thr <=> #gt>=KSEL
    for r in range(N_ROUNDS):
        wstep = W0 / (128.0 ** (r + 1))
        nc.vector.scalar_tensor_tensor(
            neg_edges, iota128, -wstep, lo, op0=ALU.mult, op1=ALU.subtract)
        nc.scalar.activation(sign_scr, lrep, AF.Sign, bias=neg_edges,
                             scale=1.0, accum_out=acc)
        nc.vector.tensor_single_scalar(sel, acc, thr_acc, op=ALU.is_ge)
        nc.gpsimd.partition_all_reduce(ssum, sel, channels=128,
                                       reduce_op=bass_isa.ReduceOp.add)
        nc.vector.scalar_tensor_tensor(
            lo, ssum, wstep, lo, op0=ALU.mult, op1=ALU.add)

    # m = (logit > lo) * sigmoid(logit)   per token, replicated on partitions
    grep = pa.tile([128, N], F32)
    nc.scalar.activation(grep, lrep, AF.Sigmoid)
    m_rep = pm.tile([128, N], F32)
    nc.vector.scalar_tensor_tensor(
        m_rep, lrep, lo, grep, op0=ALU.is_gt, op1=ALU.mult)

    # token-major copies: m_v[p, i] = m[i*128 + p]
    nc.scalar.dma_start(m_dram, m_rep[0:1, :])
    pa_ctx.close()
    pc = pm
    m_v = pc.tile([128, KC], F32)
    nc.scalar.dma_start(m_v, m_dram.rearrange("(i p) -> p i", p=128))
    if QK_FP8:
        msc_v = pc.tile([128, KC], F32)
        nc.vector.tensor_scalar(msc_v, m_v, SQK, None, op0=ALU.mult)
    else:
        msc_v = m_v

    # ---------------- compaction: selected-token index list ----------------
    # Single-partition row pipeline (pure DVE + one gpsimd scatter; nothing on
    # the PE queue, no DRAM roundtrips): csel[t] = #selected in tokens [0, t],
    # pos[t] = selected ? csel-1 : count + t - csel, clamped to < NSEL.
    m01row = pc.tile([1, N], F32)
    nc.vector.tensor_scalar(m01row, lrep[0:1, :], lo[0:1, :], None,
                            op0=ALU.is_gt)
    csel = pc.tile([1, N], F32)
    nc.vector.tensor_tensor_scan(csel, m01row, m01row, 0.0,
                                 op0=ALU.add, op1=ALU.bypass)
    r_i = pc.tile([1, N], I32)
    nc.gpsimd.iota(r_i, pattern=[[1, N]], base=0, channel_multiplier=0)
    pos = pc.tile([1, N], F32)
    nc.vector.tensor_copy(pos, r_i)           # pos <- t
    nc.vector.tensor_tensor(pos, pos, csel, op=ALU.subtract)
    nc.vector.tensor_scalar(pos, pos, csel[:, N - 1:N], None, op0=ALU.add)
    scr = pc.tile([1, N], F32)
    nc.vector.tensor_scalar(scr, csel, -1.0, None, op0=ALU.add)  # csel-1
    nc.vector.tensor_copy(r_i, m01row)        # int mask
    nc.vector.copy_predicated(pos, r_i, scr)
    # clamp: tokens landing beyond the NSEL gathered slots -> -1 (dropped)
    nc.vector.tensor_single_scalar(scr, pos, float(NSEL), op=ALU.is_lt)
    nc.vector.scalar_tensor_tensor(pos, pos, 1.0, scr,
                                   op0=ALU.add, op1=ALU.mult)
    # single-partition compaction: sc_out[0, pos[t]] = t for pos >= 0
    sc_idx = pc.tile([16, N], I16)
    nc.vector.memset(sc_idx, -1)
    nc.vector.tensor_scalar(sc_idx[0:1, :], pos, -1.0, None, op0=ALU.add)
    sc_data = pc.tile([16, N], I16)
    nc.gpsimd.iota(sc_data, pattern=[[1, N]], base=0, channel_multiplier=0)
    sc_out = pc.tile([16, NSEL], I16)
    nc.gpsimd.local_scatter(sc_out, sc_data, sc_idx, channels=16,
                            num_elems=NSEL, num_idxs=N)
    nc.scalar.dma_start(idx16_dram, sc_out[0:1, :])
    nc.scalar.dma_start(io["dbg_idx"], sc_out[0:1, :])
    nc.scalar.dma_start(io["dbg_m01"], m01row)
    # wrapped gather-index layout: idx j -> [j % 16, j // 16], replicated
    idx16_sb = big.tile([128, NSEL // 16], I16)
    for k in range(8):
        nc.scalar.dma_start(idx16_sb[16 * k:16 * (k + 1), :],
                          idx16_dram.rearrange("(s p) -> p s", p=16))
    stg_pool = pg_ctx.enter_context(tc.tile_pool(name="stg_pool", bufs=3))
    psum1 = pg_ctx.enter_context(tc.tile_pool(name="psum1", bufs=5, space="PSUM"))

    # ---------------- K projection, token-major -> DRAM rows ----------------
    pkv = pkv_ctx.enter_context(tc.tile_pool(name="pkv", bufs=1))
    wk_sb = pkv.tile([128, CC, KV_D], BF16)
    wv_sb = pkv.tile([128, CC, KV_D], BF16)
    for cc in range(CC):
        sl = slice(cc * 128, (cc + 1) * 128)
        nc.sync.dma_start(wk_sb[:, cc, :], wk[sl, :])
        nc.sync.dma_start(wv_sb[:, cc, :], wv[sl, :])
    # ---------------- V projection + gathers ----------------
    for i2 in range(KC // 2):
        vps = psum1.tile([128, 512], F32, tag="proj_ps", name=f"v_ps{i2}")
        for half in range(2):
            i = 2 * i2 + half
            ts_ = slice(i * 128, (i + 1) * 128)
            vp = vps[:, half * KV_D:(half + 1) * KV_D]
            for cc in range(CC):
                nc.tensor.matmul(vp, xb[:, cc, ts_], wv_sb[:, cc, :],
                                 start=(cc == 0), stop=(cc == CC - 1))
        vstage = stg_pool.tile([128, 2, KV_D], BF16, tag="vstage",
                               name=f"vstage{i2}")
        nc.scalar.activation(vstage[:, 0, :], vps[:, 0:KV_D], AF.Identity,
                             scale=m_v[:, 2 * i2:2 * i2 + 1])
        nc.scalar.activation(vstage[:, 1, :], vps[:, KV_D:2 * KV_D],
                             AF.Identity, scale=m_v[:, 2 * i2 + 1:2 * i2 + 2])
        nc.sync.dma_start(
            v_dram[2 * i2 * 128:(2 * i2 + 2) * 128, :]
            .rearrange("(c t) d -> t c d", t=128), vstage)
    v8g = stg_pool.tile([128, KCG, KV_D], BF16, tag="v8g")
    v8_sb = big.tile([128, KCG, HKV, 128], BF16)
    nc.vector.memset(v8_sb.bitcast(I32), 0)
    nc.vector.memset(v8_sb[:, :, :, 64:65], 1.0)
    for g in range(NSUB):
        ic = idx16_sb[:, g * (GSUB // 16):(g + 1) * (GSUB // 16)]
        cs = slice(g * (GSUB // 128), (g + 1) * (GSUB // 128))
        nc.gpsimd.dma_gather(v8g[:, cs, :], v_dram[:, :], ic,
                             GSUB, GSUB, elem_size=KV_D, transpose=False)
        nc.vector.tensor_copy(
            v8_sb[:, cs, :, 0:64],
            v8g[:, cs, :].rearrange("p c (h e) -> p c h e", e=64))

    # ---------------- K projection, token-major -> DRAM rows ----------------
    for i2 in range(KC // 2):
        kps = psum1.tile([128, 512], F32, tag="proj_ps", name=f"k_ps{i2}")
        for half in range(2):
            i = 2 * i2 + half
            ts_ = slice(i * 128, (i + 1) * 128)
            kp = kps[:, half * KV_D:(half + 1) * KV_D]
            for cc in range(CC):
                nc.tensor.matmul(kp, xb[:, cc, ts_], wk_sb[:, cc, :],
                                 start=(cc == 0), stop=(cc == CC - 1))
        kstage = stg_pool.tile([128, 2, KV_D], BF16, tag="kstage",
                               name=f"kstage{i2}")
        nc.scalar.activation(kstage[:, 0, :], kps[:, 0:KV_D], AF.Identity,
                             scale=msc_v[:, 2 * i2:2 * i2 + 1])
        nc.scalar.activation(kstage[:, 1, :], kps[:, KV_D:2 * KV_D],
                             AF.Identity,
                             scale=msc_v[:, 2 * i2 + 1:2 * i2 + 2])
        nc.sync.dma_start(
            k_dram[2 * i2 * 128:(2 * i2 + 2) * 128, :]
            .rearrange("(c t) d -> t c d", t=128), kstage)

    # ---------------- K gathers (SWDGE), pipelined into kt_z ----------------
    # dma_gather transpose mode: out[d % 128, d // 128, slot] = k[idx[slot], d]
    # -> directly the [dims-on-partitions, token-free] layout the logits
    # matmuls need.  kt_z: two parity copies with the other 64-partition half
    # zeroed, so the logits matmul contracts a full K=128 (partner q rows hit
    # zeros) and the PE HAM activity monitor stays warm.  QK_FP8 adds a
    # zeroed second DoubleRow subtile.  Sub-gathers stay well under the SWDGE
    # descriptor-ring capacity (16KB carveout = 1024 descriptors).
    kt_all = stg_pool.tile([128, NSUB, 2, GSUB], BF16, tag="kt_all")
    if QK_FP8:
        kt_z = big.tile([128, 2, 2, 2, NSEL], F8E4)   # [p, par, sub, j, t]
        nc.vector.memset(kt_z[64:128, 0, :, :, :].bitcast(I32), 0)
        nc.vector.memset(kt_z[0:64, 1, :, :, :].bitcast(I32), 0)
        nc.vector.memset(kt_z[:, :, 1, :, :].bitcast(I32), 0)
    else:
        kt_z = big.tile([128, 2, 2, NSEL], BF16)      # [p, par, j, t]
        nc.vector.memset(kt_z[64:128, 0, :, :].bitcast(I32), 0)
        nc.vector.memset(kt_z[0:64, 1, :, :].bitcast(I32), 0)
    for g in range(NSUB):
        ic = idx16_sb[:, g * (GSUB // 16):(g + 1) * (GSUB // 16)]
        nc.gpsimd.dma_gather(kt_all[:, g, :, :], k_dram[:, :], ic,
                             GSUB, GSUB, elem_size=KV_D, transpose=True)
        gs = slice(g * GSUB, (g + 1) * GSUB)
        for j in range(2):
            if QK_FP8:
                nc.vector.tensor_copy(kt_z[0:64, 0, 0, j, gs],
                                      kt_all[0:64, g, j, :])
                nc.vector.tensor_copy(kt_z[64:128, 1, 0, j, gs],
                                      kt_all[64:128, g, j, :])
            else:
                nc.vector.tensor_copy(kt_z[0:64, 0, j, gs],
                                      kt_all[0:64, g, j, :])
                nc.vector.tensor_copy(kt_z[64:128, 1, j, gs],
                                      kt_all[64:128, g, j, :])

    pkv_ctx.close()
    # ---------------- QT projection (overlaps the gather chain) ------------
    # Slot layout is permuted so each q-head lands on the same partition range
    # as its GQA kv-head in KT: head h -> slot (h%4)+4*(h//8), partition base
    # ((h//4)%2)*64.  Slot j holds heads (ha, ha+4), ha = j if j<4 else j+4.
    pq = pq_ctx.enter_context(tc.tile_pool(name="pq", bufs=1))
    wq_sb = pq.tile([128, CC, QT_D], BF16)
    for cc in range(CC):
        nc.sync.dma_start(wq_sb[:, cc, :], wq[cc * 128:(cc + 1) * 128, :])
    if QK_FP8:
        qt_sb = big.tile([128, H // 2, 2, NQ], F8E4)
        nc.vector.memset(qt_sb[:, :, 1, :].bitcast(I32), 0)
    else:
        qt_sb = big.tile([128, H // 2, NQ], BF16)
    for j in range(H // 2):
        for g in range(NQ // 512):
            ps = psum1.tile([128, 512], F32, tag="proj_ps",
                            name=f"q_ps{j}_{g}")
            qs = slice(g * 512, (g + 1) * 512)
            for cc in range(CC):
                nc.tensor.matmul(
                    ps, wq_sb[:, cc, j * 128:(j + 1) * 128],
                    xb[:, cc, qs],
                    start=(cc == 0), stop=(cc == CC - 1))
            if QK_FP8:
                nc.vector.scalar_tensor_tensor(
                    qt_sb[:, j, 0, qs], ps, SQK, m_rep[:, qs],
                    op0=ALU.mult, op1=ALU.mult)
            else:
                nc.vector.tensor_tensor(qt_sb[:, j, qs], ps, m_rep[:, qs],
                                        op=ALU.mult)
    pq_ctx.close()

    # r-route prep: fp8 V copies + exact Sum(v) over the r-chunks
    if R_PAIRS:
        psum_t = pg_ctx.enter_context(
            tc.tile_pool(name="psum_t", bufs=1, space="PSUM"))
        v8f = big.tile([128, R_PAIRS, 2, HKV, 128], F8E4)
        vsum_sb = big.tile([1, HKV, 128], BF16)
        vs_ps = psum_t.tile([1, 512], F32, tag="vs", name="vs_ps")
        for pi in range(R_PAIRS):
            for s in range(2):
                rc = P_KC + 2 * pi + s
                nc.vector.tensor_scalar(
                    v8f[:, pi, s, :, :], v8_sb[:, rc, :, :], SV, None,
                    op0=ALU.mult)
                nc.tensor.matmul(vs_ps, onescol_bf, v8_sb[:, rc, :, :],
                                 start=(pi == 0 and s == 0),
                                 stop=(pi == R_PAIRS - 1 and s == 1))
        nc.vector.tensor_copy(vsum_sb, vs_ps)
    pg_ctx.close()
    pm_ctx.close()
    px_ctx.close()  # free xT + phase-1 PSUM

    # ---------------- phase 2: attention ----------------
    ph2_ctx = contextlib.ExitStack()
    ph2 = ph2_ctx.enter_context(tc.tile_pool(name="ph2", bufs=1))
    wo_sb = ph2.tile([128, CC, C], F32R)
    for cc in range(CC):
        nc.sync.dma_start(wo_sb[:, cc, :],
                          wo[cc * 128:(cc + 1) * 128, :].bitcast(F32R))

    patt_ctx = contextlib.ExitStack()
    scr_pool = patt_ctx.enter_context(tc.tile_pool(name="scr_pool", bufs=2))
    p_pool = patt_ctx.enter_context(tc.tile_pool(name="p_pool", bufs=2))
    lg_pool = patt_ctx.enter_context(
        tc.tile_pool(name="lg_pool", bufs=2, space="PSUM"))
    att_pool = patt_ctx.enter_context(
        tc.tile_pool(name="att_pool", bufs=1, space="PSUM"))
    oT_sb = ph2.tile([128, CC, NQ], F32R)
    denom_sb = ph2.tile([16, NQ], F32)

    def lg_matmuls(lg, pair, j, kc):
        """logits for both heads of the pair into lg[0], lg[1]."""
        for m in range(2):
            jq = (pair[m] % 4) + 4 * (pair[m] // 8)
            for g in range(NQ // 512):
                gs = slice(g * 512, (g + 1) * 512)
                if QK_FP8:
                    nc.tensor.matmul(
                        lg[m][:, gs],
                        kt_z[:, m, :, j, kc * 128:(kc + 1) * 128],
                        qt_sb[:, jq, :, gs],
                        start=True, stop=True, perf_mode=DR,
                        skip_group_check=True)
                else:
                    nc.tensor.matmul(
                        lg[m][:, gs],
                        kt_z[:, m, j, kc * 128:(kc + 1) * 128],
                        qt_sb[:, jq, gs],
                        start=True, stop=True)

    pair_heads = [(ha, ha + 4) for ha in (0, 1, 2, 3, 8, 9, 10, 11)]
    for hp, pair in enumerate(pair_heads):
        j = pair[0] // 8
        att_ps = [att_pool.tile([128, NQ], F32, tag=f"att{m}", name=f"att{hp}_{m}")
                  for m in range(2)]
        pend = []  # pipelined attv matmuls: emitted one chunk behind
        for quarter in range(P_KC // 2):
            p_t = p_pool.tile([128, 2, N], BF16, tag="p_t",
                              name=f"p_{hp}_{quarter}")
            for kci in range(2):
                kc = quarter * 2 + kci
                lg = [lg_pool.tile([128, NQ], F32, tag="lg",
                                   name=f"lg{hp}_{kc}_{m2}") for m2 in range(2)]
                lg_matmuls(lg, pair, j, kc)
                for m in range(2):
                    nc.scalar.activation(
                        p_t[:, kci, m * NQ:(m + 1) * NQ], lg[m], AF.Exp,
                        scale=EXP_SCALE)
                for f in pend:
                    f()
                pend = []

                def attv(p_t=p_t, kci=kci, kc=kc):
                    for m in range(2):
                        hk = pair[m] // 4
                        for g in range(NQ // 512):
                            nc.tensor.matmul(
                                att_ps[m][:, g * 512:(g + 1) * 512],
                                v8_sb[:, kc, hk, :],
                                p_t[:, kci,
                                    m * NQ + g * 512:m * NQ + (g + 1) * 512],
                                start=(kc == 0),
                                stop=(kc == KCG - 1 and not R_PAIRS),
                                skip_group_check=True)

                pend.append(attv)
        for pi in range(R_PAIRS):
            r8 = p_pool.tile([128, 2, N], F8E4, tag="r8", name=f"r8_{hp}_{pi}")
            racc = scr_pool.tile([128, 1], F32, tag="racc", name=f"racc{hp}{pi}")
            for s in range(2):
                kc = P_KC + 2 * pi + s
                lg = [lg_pool.tile([128, NQ], F32, tag="lg",
                                   name=f"lg{hp}_{kc}_{m2}") for m2 in range(2)]
                lg_matmuls(lg, pair, j, kc)
                # r = (z + z^2/2)/SV, z = y*c: t = 1 + z/2 (DVE, psum->
                # sbuf), then (y*(c/SV))*t (one PSUM operand only)
                for m in range(2):
                    tq = scr_pool.tile([128, NQ], BF16, tag="tq",
                                       name=f"tq{hp}_{pi}_{s}_{m}")
                    nc.vector.tensor_scalar(
                        tq, lg[m], float(EXP_SCALE / 2.0), 1.0,
                        op0=ALU.mult, op1=ALU.add)
                    nc.vector.affine_mul_reduce(
                        r8[:, s, m * NQ:(m + 1) * NQ], racc, lg[m], tq,
                        scale=float(EXP_SCALE / SV), bias=0.0)
                for f in pend:
                    f()
                pend = []

            def attv_r(r8=r8, pi=pi):
                for m in range(2):
                    hk = pair[m] // 4
                    for g in range(NQ // 512):
                        nc.tensor.matmul(
                            att_ps[m][:, g * 512:(g + 1) * 512],
                            v8f[:, pi, :, hk, :],
                            r8[:, :, m * NQ + g * 512:m * NQ + (g + 1) * 512],
                            start=False, stop=False, perf_mode=DR,
                            skip_group_check=True)

            pend.append(attv_r)
        for f in pend:
            f()
        if R_PAIRS:
            # rank-1 correction: + Sum_{r-chunks} v (incl. slot count in the
            # ones column) to every query column
            for m in range(2):
                hk = pair[m] // 4
                for g in range(NQ // 512):
                    nc.tensor.matmul(
                        att_ps[m][:, g * 512:(g + 1) * 512],
                        vsum_sb[0:1, hk, :], ones512,
                        start=False, stop=True, skip_group_check=True)
        # fast evict: copy [65, NQ] psum -> sbuf scratch, stash denom row,
        # numerator into oT unscaled; 1/denom once after all pairs.
        for m in range(2):
            h = pair[m]
            scr65 = scr_pool.tile([65, NQ], F32R, tag="scr65",
                                  name=f"scr65_{hp}_{m}")
            nc.vector.tensor_copy(scr65, att_ps[m][0:65, :])
            nc.sync.dma_start(denom_sb[h:h + 1, :],
                              scr65[64:65, :].bitcast(F32))
            if h % 2 == 0:
                nc.vector.tensor_copy(oT_sb[0:64, h // 2, :], scr65[0:64, :])
            else:
                # partition shift 0 -> 64 must go through DMA
                nc.sync.dma_start(oT_sb[64:128, h // 2, :], scr65[0:64, :])

    # denominator: + (N - NSEL) for the never-gathered masked keys, then
    # one batched reciprocal and a per-d-chunk broadcast multiply.
    nc.vector.tensor_scalar(denom_sb, denom_sb, float(N - NSEL), None,
                            op0=ALU.add)
    rec16 = ph2.tile([16, NQ], F32R)
    rec16_f = ph2.tile([16, NQ], F32)
    with nc.allow_low_precision(reason="2e-5 rel err << output tolerance"):
        nc.vector.reciprocal_approx_fast(out=rec16_f, in_=denom_sb)
    nc.vector.tensor_copy(rec16, rec16_f)
    for dd in range(CC):
        for g in range(NQ // 512):
            bps = lg_pool.tile([128, 512], F32, tag="lg", name=f"bps{dd}_{g}")
            nc.tensor.matmul(
                bps, sel8[:, dd, :], rec16[:, g * 512:(g + 1) * 512],
                start=True, stop=True)
            sl = slice(g * 512, (g + 1) * 512)
            nc.vector.tensor_tensor(
                oT_sb[:, dd, sl], oT_sb[:, dd, sl], bps, op=ALU.mult)
    patt_ctx.close()
    # ---------------- phase 3: output projection ----------------
    ph3_ctx = contextlib.ExitStack()
    psum3 = ph3_ctx.enter_context(tc.tile_pool(name="psum3", bufs=4, space="PSUM"))
    out_pool = ph3_ctx.enter_context(tc.tile_pool(name="out_pool", bufs=2))
    for tt in range(NQ // 128):
        out_sb = out_pool.tile([128, C], F32, tag="out_sb", name=f"out_sb{tt}")
        for og in range(C // 512):
            ps = psum3.tile([128, 512], F32, tag="out_ps", name=f"out_ps{tt}_{og}")
            for dd in range(CC):
                nc.tensor.matmul(
                    ps, oT_sb[:, dd, tt * 128:(tt + 1) * 128],
                    wo_sb[:, dd, og * 512:(og + 1) * 512],
                    start=(dd == 0), stop=(dd == CC - 1))
            nc.scalar.copy(out_sb[:, og * 512:(og + 1) * 512], ps)
        nc.sync.dma_start(out_d[tt * 128:(tt + 1) * 128, :], out_sb)
    ph3_ctx.close()
    ph2_ctx.close()


_NC = None


def build_program():
    global _NC
    if _NC is not None:
        return _NC
    from contextlib import ExitStack

    nc = bacc.Bacc("TRN2", target_bir_lowering=False, debug=False, num_devices=8)
    io = {
        "xT": nc.dram_tensor("xT", (C, N), F32, kind="ExternalInput").ap(),
        "wq": nc.dram_tensor("wq", (C, QT_D), BF16, kind="ExternalInput").ap(),
        "wk": nc.dram_tensor("wk", (C, KV_D), BF16, kind="ExternalInput").ap(),
        "wv": nc.dram_tensor("wv", (C, KV_D), BF16, kind="ExternalInput").ap(),
        "rw": nc.dram_tensor("rw", (C, 1), F32, kind="ExternalInput").ap(),
        "wo": nc.dram_tensor("wo", (C, C), F32, kind="ExternalInput").ap(),
        "sel8": nc.dram_tensor("sel8", (16, CC, 128), F32,
                               kind="ExternalInput").ap(),
        "out": nc.dram_tensor("out", (NQ, C), F32, kind="ExternalOutput").ap(),
        "dbg_idx": nc.dram_tensor("dbg_idx", (NSEL,), I16,
                                  kind="ExternalOutput").ap(),
        "dbg_m01": nc.dram_tensor("dbg_m01", (N,), F32,
                                  kind="ExternalOutput").ap(),
    }
    with TileContext(nc) as tc:
        with ExitStack() as ctx:
            _emit(nc, tc, ctx, io)
    nc.compile()
    _NC = nc
    return nc


def _permute_wq(wq):
    """Column-permute wq so QT slot j's 128 cols = heads (ha, ha+4) contig."""
    wq = np.asarray(wq, np.float32).reshape(C, H, DH)
    order = []
    for j in range(H // 2):
        ha = j if j < 4 else j + 4
        order += [ha, ha + 4]
    return np.ascontiguousarray(wq[:, order, :].reshape(C, H * DH))


def make_in_maps(x, router_w, wq, wk, wv, wo):
    wq = _permute_wq(wq)
    in_maps = []
    for core in range(8):
        b, h = core // 2, core % 2
        xT_core = np.ascontiguousarray(
            np.roll(np.asarray(x[b], np.float32).T, -h * NQ, axis=1))
        sel8 = np.zeros((16, CC, 128), np.float32)
        for dd in range(CC):
            for p in range(128):
                sel8[2 * dd + p // 64, dd, p] = 1.0
        in_maps.append({
            "xT": xT_core,
            "sel8": sel8,
            "wq": np.ascontiguousarray(np.asarray(wq, np.float32),
                                       dtype=ml_dtypes.bfloat16),
            "wk": np.ascontiguousarray(np.asarray(wk, np.float32),
                                       dtype=ml_dtypes.bfloat16),
            "wv": np.ascontiguousarray(np.asarray(wv, np.float32),
                                       dtype=ml_dtypes.bfloat16),
            "rw": np.ascontiguousarray(router_w, dtype=np.float32),
            "wo": np.ascontiguousarray(wo, dtype=np.float32),
        })
    return in_maps


def _numpy_fallback(x, router_w, router_b, wq, bq, wk, bk, wv, bv, wo, bo):
    x = np.asarray(x, np.float32)
    gate = 1.0 / (1.0 + np.exp(-(x @ router_w + router_b)))
    xg = x * gate
    scores = gate[..., 0]
    idx = np.argsort(-scores, axis=-1, kind="stable")[:, :KSEL]
    mask = np.zeros((x.shape[0], x.shape[1]), np.float32)
    np.put_along_axis(mask, idx, 1.0, axis=1)
    xg = xg * mask[..., None]
    q = (xg @ wq + bq).reshape(B, N, H, DH)
    kk = np.repeat((xg @ wk + bk).reshape(B, N, HKV, DH), H // HKV, axis=2)
    v = np.repeat((xg @ wv + bv).reshape(B, N, HKV, DH), H // HKV, axis=2)
    att = np.einsum("bqhd,bkhd->bhqk", q, kk) / np.float32(np.sqrt(DH))
    att = att - att.max(-1, keepdims=True)
    att = np.exp(att)
    att = att / att.sum(-1, keepdims=True)
    o = np.einsum("bhqk,bkhd->bqhd", att, v).reshape(B, N, C)
    return (o @ wo + bo).astype(np.float32)


def kernel(x, router_w, router_b, wq, bq, wk, bk, wv, bv, wo, bo):
    x = np.asarray(x)
    biases = [router_b, bq, bk, bv, bo]
    if any(float(np.abs(np.asarray(t)).max()) != 0.0 for t in biases):
        # The device program folds away the (identically zero) biases; fall
        # back to an exact host implementation if that assumption breaks.
        return _numpy_fallback(x, router_w, router_b, wq, bq, wk, bk, wv, bv,
                               wo, bo)

    from concourse import bass_utils

    nc = build_program()
    in_maps = make_in_maps(x, router_w, wq, wk, wv, wo)
    res = bass_utils.run_bass_kernel_spmd(nc, in_maps, core_ids=list(range(8)))
    out = np.empty((B, N, C), np.float32)
    for core in range(8):
        b, h = core // 2, core % 2
        out[b, h * NQ:(h + 1) * NQ, :] = res.results[core]["out"]
    return out



# revision 28
# speedup vs baseline: 1.3238x; 1.1680x over previous
"""Trainium2 Bass kernel for MIGAttention (topk token masking + GQA attention).

Shapes (hardcoded): B=4, N=2048, C=1024, H=16 heads, HKV=4 kv-heads, DH=64,
keep-ratio 0.7 -> k = 1433 selected tokens per batch row.

Sharding: 8 cores = (batch b in 0..3) x (query-half h in 0..1).  Each core
receives x[b].T with token columns rolled by h*1024 so that its own query
half always occupies columns 0..1023 -> a single SPMD program for all cores.

Key structure (v2): the top-k mask selects 1433 of 2048 tokens; masked tokens
have zero K/V rows.  The kernel compacts the selected tokens on device
(prefix-sum + indirect-DMA scatter/gather) and runs attention over only
NSEL=1536 gathered key slots (12 chunks instead of 16).  Junk tail slots
(gathered masked tokens) have zero K/V so their logits are exactly 0 and
p=exp(0)=1, standing in one-for-one for masked tokens in the softmax
denominator; the remaining 2048-1536=512 masked keys are a static +512
constant added to the denominator.

QK_FP8: Q/K are quantized to fp8e4 at the projection evict and the logits
matmuls run in DoubleRow perf mode (2x).  R_PAIRS: the last 2*R_PAIRS key
chunks compute r = expm1(y) ~ y + y^2/2 on the Vector engine (one fused
affine_mul op straight from PSUM) in fp8, and their att@V uses fp8 DoubleRow
with the exact Sum(v) rank-1 correction folded into the same PSUM
accumulation.  This keeps all fp8 error on the small residual r (rms ~0.1)
instead of p (~1.0).
"""

import contextlib
import sys

import ml_dtypes
import numpy as np

if "/opt/trn_rl_repo" not in sys.path:
    sys.path.insert(0, "/opt/trn_rl_repo")

import concourse.bass as bass  # noqa: F401
import concourse.bass_isa as bass_isa
import concourse.mybir as mybir
from concourse import bacc
from concourse.tile import TileContext

F32 = mybir.dt.float32
F32R = mybir.dt.float32r
BF16 = mybir.dt.bfloat16
F8E4 = mybir.dt.float8e4
I32 = mybir.dt.int32
I16 = mybir.dt.int16
AF = mybir.ActivationFunctionType
ALU = mybir.AluOpType
DR = mybir.MatmulPerfMode.DoubleRow

B, N, C = 4, 2048, 1024
H, HKV, DH = 16, 4, 64
NQ = N // 2          # queries per core
KSEL = 1433          # max(1, int(N * 0.7))
CC = C // 128        # contraction chunks (8)
KC = N // 128        # dense token chunks (16)
KCG = 12             # gathered key chunks (1536 slots >= KSEL)
NSEL = KCG * 128     # 1536
QT_D = H * DH        # 1024
KV_D = HKV * DH      # 256
N_ROUNDS = 4         # topk threshold refinement rounds (8/128^4 ~ 3e-8 << min topk gap 3.5e-6)
LO0, W0 = -4.0, 8.0  # initial logit search interval (logit std ~0.65)

# ---- tuning flags ----
QK_FP8 = True        # fp8 Q/K + DoubleRow logits matmuls
R_PAIRS = 0          # of the 12 chunks, last 2*R_PAIRS use the DVE r-route
P_KC = KCG - 2 * R_PAIRS
SQK = 32.0           # fp8 quantization scale for q/k
SV = 4.0             # fp8 v scale for the r-route (r is divided by SV)
EXP_SCALE = 1.0 / (np.sqrt(DH) * (SQK * SQK if QK_FP8 else 1.0))
KDT = F8E4 if QK_FP8 else BF16
GSUB, NSUB = 384, NSEL // 384  # SWDGE sub-gather split


def _emit(nc, tc, ctx, io):
    xT, wq, wk, wv, rw, wo, out_d = (
        io["xT"], io["wq"], io["wk"], io["wv"], io["rw"], io["wo"], io["out"])

    # ---------------- long-lived pools ----------------
    const = ctx.enter_context(tc.tile_pool(name="const", bufs=1))
    small = ctx.enter_context(tc.tile_pool(name="small", bufs=1))
    big = ctx.enter_context(tc.tile_pool(name="big", bufs=1))
    dram = ctx.enter_context(tc.tile_pool(name="dram", bufs=1, space="DRAM"))

    # nesting (LIFO): px > pm > {pa, pcs, pg > {pq, pkv}}
    px_ctx = contextlib.ExitStack()   # xT (alive through projections+gather)
    pm_ctx = contextlib.ExitStack()   # m_rep/m_v/compaction sbuf scratch
    pa_ctx = contextlib.ExitStack()   # router/refinement scratch
    pg_ctx = contextlib.ExitStack()   # proj staging + transpose psum
    pq_ctx = contextlib.ExitStack()   # wq
    pkv_ctx = contextlib.ExitStack()  # wk, wv
    px = px_ctx.enter_context(tc.tile_pool(name="px", bufs=1))
    pm = pm_ctx.enter_context(tc.tile_pool(name="pm", bufs=1))
    pa = pa_ctx.enter_context(tc.tile_pool(name="pa", bufs=1))
    psum_r = pa_ctx.enter_context(tc.tile_pool(name="psum_r", bufs=1, space="PSUM"))

    # ---------------- constants ----------------
    ones_row = const.tile([1, 128], F32)
    nc.vector.memset(ones_row, 1.0)
    ones512 = const.tile([1, 512], BF16)
    nc.vector.memset(ones512, 1.0)
    onescol_bf = const.tile([128, 1], BF16)
    nc.vector.memset(onescol_bf, 1.0)
    iota128_i = const.tile([128, 1], I32)
    nc.gpsimd.iota(iota128_i, pattern=[[0, 1]], base=1, channel_multiplier=1)
    iota128 = const.tile([128, 1], F32)
    nc.vector.tensor_copy(iota128, iota128_i)
    sel8 = const.tile([16, CC, 128], F32R)
    nc.sync.dma_start(sel8, io["sel8"].bitcast(F32R))

    # ---------------- DRAM scratch ----------------
    m_dram = dram.tile([N], F32)
    idx16_dram = dram.tile([NSEL], I16)
    k_dram = dram.tile([N, KV_D], BF16)
    v_dram = dram.tile([N, KV_D], BF16)

    # ---------------- router: logits = x @ rw, exact fp32 ----------------
    rw_sb = pa.tile([128, CC], F32)
    for cc in range(CC):
        sl = slice(cc * 128, (cc + 1) * 128)
        nc.sync.dma_start(rw_sb[:, cc:cc + 1], rw[sl, :])
    # Single fp32 x load feeds both the exact-fp32 PE router matmuls (the
    # instruction structure must match the reference summation closely: batch
    # 0's topk threshold gap is 3.5e-6, so any other reduction order flips a
    # near-tie - measured both for f32r and for a DVE-partials variant) and,
    # via engine casts, the bf16 working copy the projections use.
    logits_sb = pa.tile([1, N], F32)
    xr_pool = pa_ctx.enter_context(tc.tile_pool(name="xr_pool", bufs=3))
    xb = px.tile([128, CC, N], BF16)
    rps = [psum_r.tile([1, 512], F32, tag=f"router_ps{g}", name=f"router_ps{g}")
           for g in range(4)]
    for cc in range(CC):
        xr = xr_pool.tile([128, N], F32, tag="xr", name=f"xr{cc}")
        eng = nc.sync if cc % 2 == 0 else nc.scalar
        eng.dma_start(xr, xT[cc * 128:(cc + 1) * 128, :])
        for g in range(4):
            nc.tensor.matmul(
                rps[g], rw_sb[:, cc:cc + 1], xr[:, g * 512:(g + 1) * 512],
                start=(cc == 0), stop=(cc == CC - 1))
        if cc % 2 == 0:
            nc.scalar.copy(xb[:, cc, :], xr)
        else:
            nc.vector.tensor_copy(xb[:, cc, :], xr)
    for g in range(4):
        nc.vector.tensor_copy(logits_sb[:, g * 512:(g + 1) * 512], rps[g])

    # replicate logits across all 128 partitions (K=1 matmul broadcast)
    lrep = pa.tile([128, N], F32)
    for g in range(4):
        ps = psum_r.tile([128, 512], F32, tag="bcast_ps")
        nc.tensor.matmul(ps, ones_row, logits_sb[:, g * 512:(g + 1) * 512],
                         start=True, stop=True)
        nc.vector.tensor_copy(lrep[:, g * 512:(g + 1) * 512], ps)

    # ---------------- topk threshold refinement ----------------
    # invariant: v* (the KSEL-th largest logit) is in (lo, lo + w]
    lo = small.tile([128, 1], F32)
    nc.vector.memset(lo, LO0)
    neg_edges = small.tile([128, 1], F32)
    acc = small.tile([128, 1], F32)
    sel = small.tile([128, 1], F32)
    ssum = small.tile([128, 1], F32)
    sign_scr = pa.tile([128, N], BF16)  # Sign output is never read
    thr_acc = float(2 * KSEL - N)  # acc = #gt - #lt ; acc>=thr <=> #gt>=KSEL
    for r in range(N_ROUNDS):
        wstep = W0 / (128.0 ** (r + 1))
        nc.vector.scalar_tensor_tensor(
            neg_edges, iota128, -wstep, lo, op0=ALU.mult, op1=ALU.subtract)
        nc.scalar.activation(sign_scr, lrep, AF.Sign, bias=neg_edges,
                             scale=1.0, accum_out=acc)
        nc.vector.tensor_single_scalar(sel, acc, thr_acc, op=ALU.is_ge)
        nc.gpsimd.partition_all_reduce(ssum, sel, channels=128,
                                       reduce_op=bass_isa.ReduceOp.add)
        nc.vector.scalar_tensor_tensor(
            lo, ssum, wstep, lo, op0=ALU.mult, op1=ALU.add)

    # m = (logit > lo) * sigmoid(logit)   per token, replicated on partitions
    grep = pa.tile([128, N], F32)
    nc.scalar.activation(grep, lrep, AF.Sigmoid)
    m_rep = pm.tile([128, N], F32)
    nc.vector.scalar_tensor_tensor(
        m_rep, lrep, lo, grep, op0=ALU.is_gt, op1=ALU.mult)

    # token-major copies: m_v[p, i] = m[i*128 + p]
    nc.scalar.dma_start(m_dram, m_rep[0:1, :])
    pa_ctx.close()
    pc = pm
    m_v = pc.tile([128, KC], F32)
    nc.scalar.dma_start(m_v, m_dram.rearrange("(i p) -> p i", p=128))
    if QK_FP8:
        msc_v = pc.tile([128, KC], F32)
        nc.vector.tensor_scalar(msc_v, m_v, SQK, None, op0=ALU.mult)
    else:
        msc_v = m_v

    # ---------------- compaction: selected-token index list ----------------
    # Single-partition row pipeline (pure DVE + one gpsimd scatter; nothing on
    # the PE queue, no DRAM roundtrips): csel[t] = #selected in tokens [0, t],
    # pos[t] = selected ? csel-1 : count + t - csel, clamped to < NSEL.
    m01row = pc.tile([1, N], F32)
    nc.vector.tensor_scalar(m01row, lrep[0:1, :], lo[0:1, :], None,
                            op0=ALU.is_gt)
    csel = pc.tile([1, N], F32)
    nc.vector.tensor_tensor_scan(csel, m01row, m01row, 0.0,
                                 op0=ALU.add, op1=ALU.bypass)
    r_i = pc.tile([1, N], I32)
    nc.gpsimd.iota(r_i, pattern=[[1, N]], base=0, channel_multiplier=0)
    pos = pc.tile([1, N], F32)
    nc.vector.tensor_copy(pos, r_i)           # pos <- t
    nc.vector.tensor_tensor(pos, pos, csel, op=ALU.subtract)
    nc.vector.tensor_scalar(pos, pos, csel[:, N - 1:N], None, op0=ALU.add)
    scr = pc.tile([1, N], F32)
    nc.vector.tensor_scalar(scr, csel, -1.0, None, op0=ALU.add)  # csel-1
    nc.vector.tensor_copy(r_i, m01row)        # int mask
    nc.vector.copy_predicated(pos, r_i, scr)
    # clamp: tokens landing beyond the NSEL gathered slots -> -1 (dropped)
    nc.vector.tensor_single_scalar(scr, pos, float(NSEL), op=ALU.is_lt)
    nc.vector.scalar_tensor_tensor(pos, pos, 1.0, scr,
                                   op0=ALU.add, op1=ALU.mult)
    # single-partition compaction: sc_out[0, pos[t]] = t for pos >= 0
    sc_idx = pc.tile([16, N], I16)
    nc.vector.memset(sc_idx, -1)
    nc.vector.tensor_scalar(sc_idx[0:1, :], pos, -1.0, None, op0=ALU.add)
    sc_data = pc.tile([16, N], I16)
    nc.gpsimd.iota(sc_data, pattern=[[1, N]], base=0, channel_multiplier=0)
    sc_out = pc.tile([16, NSEL], I16)
    nc.gpsimd.local_scatter(sc_out, sc_data, sc_idx, channels=16,
                            num_elems=NSEL, num_idxs=N)
    nc.scalar.dma_start(idx16_dram, sc_out[0:1, :])
    nc.scalar.dma_start(io["dbg_idx"], sc_out[0:1, :])
    nc.scalar.dma_start(io["dbg_m01"], m01row)
    # wrapped gather-index layout: idx j -> [j % 16, j // 16], replicated
    idx16_sb = big.tile([128, NSEL // 16], I16)
    for k in range(8):
        nc.scalar.dma_start(idx16_sb[16 * k:16 * (k + 1), :],
                          idx16_dram.rearrange("(s p) -> p s", p=16))
    stg_pool = pg_ctx.enter_context(tc.tile_pool(name="stg_pool", bufs=3))
    psum1 = pg_ctx.enter_context(tc.tile_pool(name="psum1", bufs=5, space="PSUM"))

    # ---------------- K projection, token-major -> DRAM rows ----------------
    pkv = pkv_ctx.enter_context(tc.tile_pool(name="pkv", bufs=1))
    wk_sb = pkv.tile([128, CC, KV_D], BF16)
    wv_sb = pkv.tile([128, CC, KV_D], BF16)
    for cc in range(CC):
        sl = slice(cc * 128, (cc + 1) * 128)
        nc.sync.dma_start(wk_sb[:, cc, :], wk[sl, :])
        nc.sync.dma_start(wv_sb[:, cc, :], wv[sl, :])
    # ---------------- V projection + gathers ----------------
    # ---------------- K projection, token-major -> DRAM rows ----------------
    pkv = pkv_ctx.enter_context(tc.tile_pool(name="pkv", bufs=1))
    wk_sb = pkv.tile([128, CC, KV_D], BF16)
    wv_sb = pkv.tile([128, CC, KV_D], BF16)
    for cc in range(CC):
        sl = slice(cc * 128, (cc + 1) * 128)
        nc.sync.dma_start(wk_sb[:, cc, :], wk[sl, :])
        nc.sync.dma_start(wv_sb[:, cc, :], wv[sl, :])
    # ---------------- V projection + gathers ----------------
    for i2 in range(KC // 2):
        vps = psum1.tile([128, 512], F32, tag="proj_ps", name=f"v_ps{i2}")
        for half in range(2):
            i = 2 * i2 + half
            ts_ = slice(i * 128, (i + 1) * 128)
            vp = vps[:, half * KV_D:(half + 1) * KV_D]
            for cc in range(CC):
                nc.tensor.matmul(vp, xb[:, cc, ts_], wv_sb[:, cc, :],
                                 start=(cc == 0), stop=(cc == CC - 1))
        vstage = stg_pool.tile([128, 2, KV_D], BF16, tag="vstage",
                               name=f"vstage{i2}")
        nc.scalar.activation(vstage[:, 0, :], vps[:, 0:KV_D], AF.Identity,
                             scale=m_v[:, 2 * i2:2 * i2 + 1])
        nc.scalar.activation(vstage[:, 1, :], vps[:, KV_D:2 * KV_D],
                             AF.Identity, scale=m_v[:, 2 * i2 + 1:2 * i2 + 2])
        nc.sync.dma_start(
            v_dram[2 * i2 * 128:(2 * i2 + 2) * 128, :]
            .rearrange("(c t) d -> t c d", t=128), vstage)
    v8g = stg_pool.tile([128, KCG, KV_D], BF16, tag="v8g")
    v8_sb = big.tile([128, KCG, HKV, 128], BF16)
    nc.vector.memset(v8_sb.bitcast(I32), 0)
    nc.vector.memset(v8_sb[:, :, :, 64:65], 1.0)
    for g in range(NSUB):
        ic = idx16_sb[:, g * (GSUB // 16):(g + 1) * (GSUB // 16)]
        cs = slice(g * (GSUB // 128), (g + 1) * (GSUB // 128))
        nc.gpsimd.dma_gather(v8g[:, cs, :], v_dram[:, :], ic,
                             GSUB, GSUB, elem_size=KV_D, transpose=False)
        nc.vector.tensor_copy(
            v8_sb[:, cs, :, 0:64],
            v8g[:, cs, :].rearrange("p c (h e) -> p c h e", e=64))

    # ---------------- K projection, token-major -> DRAM rows ----------------
    for i2 in range(KC // 2):
        kps = psum1.tile([128, 512], F32, tag="proj_ps", name=f"k_ps{i2}")
        for half in range(2):
            i = 2 * i2 + half
            ts_ = slice(i * 128, (i + 1) * 128)
            kp = kps[:, half * KV_D:(half + 1) * KV_D]
            for cc in range(CC):
                nc.tensor.matmul(kp, xb[:, cc, ts_], wk_sb[:, cc, :],
                                 start=(cc == 0), stop=(cc == CC - 1))
        kstage = stg_pool.tile([128, 2, KV_D], BF16, tag="kstage",
                               name=f"kstage{i2}")
        nc.scalar.activation(kstage[:, 0, :], kps[:, 0:KV_D], AF.Identity,
                             scale=msc_v[:, 2 * i2:2 * i2 + 1])
        nc.scalar.activation(kstage[:, 1, :], kps[:, KV_D:2 * KV_D],
                             AF.Identity,
                             scale=msc_v[:, 2 * i2 + 1:2 * i2 + 2])
        nc.sync.dma_start(
            k_dram[2 * i2 * 128:(2 * i2 + 2) * 128, :]
            .rearrange("(c t) d -> t c d", t=128), kstage)

    # ---------------- K gathers (SWDGE), pipelined into kt_z ----------------
    # dma_gather transpose mode: out[d % 128, d // 128, slot] = k[idx[slot], d]
    # -> directly the [dims-on-partitions, token-free] layout the logits
    # matmuls need.  kt_z: two parity copies with the other 64-partition half
    # zeroed, so the logits matmul contracts a full K=128 (partner q rows hit
    # zeros) and the PE HAM activity monitor stays warm.  QK_FP8 adds a
    # zeroed second DoubleRow subtile.  Sub-gathers stay well under the SWDGE
    # descriptor-ring capacity (16KB carveout = 1024 descriptors).
    kt_all = stg_pool.tile([128, NSUB, 2, GSUB], BF16, tag="kt_all")
    if QK_FP8:
        kt_z = big.tile([128, 2, 2, 2, NSEL], F8E4)   # [p, par, sub, j, t]
        nc.vector.memset(kt_z[64:128, 0, :, :, :].bitcast(I32), 0)
        nc.vector.memset(kt_z[0:64, 1, :, :, :].bitcast(I32), 0)
        nc.vector.memset(kt_z[:, :, 1, :, :].bitcast(I32), 0)
    else:
        kt_z = big.tile([128, 2, 2, NSEL], BF16)      # [p, par, j, t]
        nc.vector.memset(kt_z[64:128, 0, :, :].bitcast(I32), 0)
        nc.vector.memset(kt_z[0:64, 1, :, :].bitcast(I32), 0)
    for g in range(NSUB):
        ic = idx16_sb[:, g * (GSUB // 16):(g + 1) * (GSUB // 16)]
        nc.gpsimd.dma_gather(kt_all[:, g, :, :], k_dram[:, :], ic,
                             GSUB, GSUB, elem_size=KV_D, transpose=True)
        gs = slice(g * GSUB, (g + 1) * GSUB)
        for j in range(2):
            if QK_FP8:
                nc.vector.tensor_copy(kt_z[0:64, 0, 0, j, gs],
                                      kt_all[0:64, g, j, :])
                nc.vector.tensor_copy(kt_z[64:128, 1, 0, j, gs],
                                      kt_all[64:128, g, j, :])
            else:
                nc.vector.tensor_copy(kt_z[0:64, 0, j, gs],
                                      kt_all[0:64, g, j, :])
                nc.vector.tensor_copy(kt_z[64:128, 1, j, gs],
                                      kt_all[64:128, g, j, :])

    for i2 in range(KC // 2):
        vps = psum1.tile([128, 512], F32, tag="proj_ps", name=f"v_ps{i2}")
        for half in range(2):
            i = 2 * i2 + half
            ts_ = slice(i * 128, (i + 1) * 128)
            vp = vps[:, half * KV_D:(half + 1) * KV_D]
            for cc in range(CC):
                nc.tensor.matmul(vp, xb[:, cc, ts_], wv_sb[:, cc, :],
                                 start=(cc == 0), stop=(cc == CC - 1))
        vstage = stg_pool.tile([128, 2, KV_D], BF16, tag="vstage",
                               name=f"vstage{i2}")
        nc.scalar.activation(vstage[:, 0, :], vps[:, 0:KV_D], AF.Identity,
                             scale=m_v[:, 2 * i2:2 * i2 + 1])
        nc.scalar.activation(vstage[:, 1, :], vps[:, KV_D:2 * KV_D],
                             AF.Identity, scale=m_v[:, 2 * i2 + 1:2 * i2 + 2])
        nc.sync.dma_start(
            v_dram[2 * i2 * 128:(2 * i2 + 2) * 128, :]
            .rearrange("(c t) d -> t c d", t=128), vstage)
    pkv_ctx.close()
    # ---------------- QT projection (overlaps the gather chain) ------------
    # Slot layout is permuted so each q-head lands on the same partition range
    # as its GQA kv-head in KT: head h -> slot (h%4)+4*(h//8), partition base
    # ((h//4)%2)*64.  Slot j holds heads (ha, ha+4), ha = j if j<4 else j+4.
    pq = pq_ctx.enter_context(tc.tile_pool(name="pq", bufs=1))
    wq_sb = pq.tile([128, CC, QT_D], BF16)
    for cc in range(CC):
        nc.sync.dma_start(wq_sb[:, cc, :], wq[cc * 128:(cc + 1) * 128, :])
    if QK_FP8:
        qt_sb = big.tile([128, H // 2, 2, NQ], F8E4)
        nc.vector.memset(qt_sb[:, :, 1, :].bitcast(I32), 0)
    else:
        qt_sb = big.tile([128, H // 2, NQ], BF16)
    for j in range(H // 2):
        for g in range(NQ // 512):
            ps = psum1.tile([128, 512], F32, tag="proj_ps",
                            name=f"q_ps{j}_{g}")
            qs = slice(g * 512, (g + 1) * 512)
            for cc in range(CC):
                nc.tensor.matmul(
                    ps, wq_sb[:, cc, j * 128:(j + 1) * 128],
                    xb[:, cc, qs],
                    start=(cc == 0), stop=(cc == CC - 1))
            if QK_FP8:
                nc.vector.scalar_tensor_tensor(
                    qt_sb[:, j, 0, qs], ps, SQK, m_rep[:, qs],
                    op0=ALU.mult, op1=ALU.mult)
            else:
                nc.vector.tensor_tensor(qt_sb[:, j, qs], ps, m_rep[:, qs],
                                        op=ALU.mult)
    pq_ctx.close()

    # r-route prep: fp8 V copies + exact Sum(v) over the r-chunks
    if R_PAIRS:
        psum_t = pg_ctx.enter_context(
            tc.tile_pool(name="psum_t", bufs=1, space="PSUM"))
        v8f = big.tile([128, R_PAIRS, 2, HKV, 128], F8E4)
        vsum_sb = big.tile([1, HKV, 128], BF16)
        vs_ps = psum_t.tile([1, 512], F32, tag="vs", name="vs_ps")
        for pi in range(R_PAIRS):
            for s in range(2):
                rc = P_KC + 2 * pi + s
                nc.vector.tensor_scalar(
                    v8f[:, pi, s, :, :], v8_sb[:, rc, :, :], SV, None,
                    op0=ALU.mult)
                nc.tensor.matmul(vs_ps, onescol_bf, v8_sb[:, rc, :, :],
                                 start=(pi == 0 and s == 0),
                                 stop=(pi == R_PAIRS - 1 and s == 1))
        nc.vector.tensor_copy(vsum_sb, vs_ps)
    pg_ctx.close()
    pm_ctx.close()
    px_ctx.close()  # free xT + phase-1 PSUM

    # ---------------- phase 2: attention ----------------
    ph2_ctx = contextlib.ExitStack()
    ph2 = ph2_ctx.enter_context(tc.tile_pool(name="ph2", bufs=1))
    wo_sb = ph2.tile([128, CC, C], F32R)
    for cc in range(CC):
        nc.sync.dma_start(wo_sb[:, cc, :],
                          wo[cc * 128:(cc + 1) * 128, :].bitcast(F32R))

    patt_ctx = contextlib.ExitStack()
    scr_pool = patt_ctx.enter_context(tc.tile_pool(name="scr_pool", bufs=2))
    p_pool = patt_ctx.enter_context(tc.tile_pool(name="p_pool", bufs=2))
    lg_pool = patt_ctx.enter_context(
        tc.tile_pool(name="lg_pool", bufs=2, space="PSUM"))
    att_pool = patt_ctx.enter_context(
        tc.tile_pool(name="att_pool", bufs=1, space="PSUM"))
    oT_sb = ph2.tile([128, CC, NQ], F32R)
    denom_sb = ph2.tile([16, NQ], F32)

    def lg_matmuls(lg, pair, j, kc):
        """logits for both heads of the pair into lg[0], lg[1]."""
        for m in range(2):
            jq = (pair[m] % 4) + 4 * (pair[m] // 8)
            for g in range(NQ // 512):
                gs = slice(g * 512, (g + 1) * 512)
                if QK_FP8:
                    nc.tensor.matmul(
                        lg[m][:, gs],
                        kt_z[:, m, :, j, kc * 128:(kc + 1) * 128],
                        qt_sb[:, jq, :, gs],
                        start=True, stop=True, perf_mode=DR,
                        skip_group_check=True)
                else:
                    nc.tensor.matmul(
                        lg[m][:, gs],
                        kt_z[:, m, j, kc * 128:(kc + 1) * 128],
                        qt_sb[:, jq, gs],
                        start=True, stop=True)

    pair_heads = [(ha, ha + 4) for ha in (0, 1, 2, 3, 8, 9, 10, 11)]
    for hp, pair in enumerate(pair_heads):
        j = pair[0] // 8
        att_ps = [att_pool.tile([128, NQ], F32, tag=f"att{m}", name=f"att{hp}_{m}")
                  for m in range(2)]
        pend = []  # pipelined attv matmuls: emitted one chunk behind
        for quarter in range(P_KC // 2):
            p_t = p_pool.tile([128, 2, N], BF16, tag="p_t",
                              name=f"p_{hp}_{quarter}")
            for kci in range(2):
                kc = quarter * 2 + kci
                lg = [lg_pool.tile([128, NQ], F32, tag="lg",
                                   name=f"lg{hp}_{kc}_{m2}") for m2 in range(2)]
                lg_matmuls(lg, pair, j, kc)
                for m in range(2):
                    nc.scalar.activation(
                        p_t[:, kci, m * NQ:(m + 1) * NQ], lg[m], AF.Exp,
                        scale=EXP_SCALE)
                for f in pend:
                    f()
                pend = []

                def attv(p_t=p_t, kci=kci, kc=kc):
                    for m in range(2):
                        hk = pair[m] // 4
                        for g in range(NQ // 512):
                            nc.tensor.matmul(
                                att_ps[m][:, g * 512:(g + 1) * 512],
                                v8_sb[:, kc, hk, :],
                                p_t[:, kci,
                                    m * NQ + g * 512:m * NQ + (g + 1) * 512],
                                start=(kc == 0),
                                stop=(kc == KCG - 1 and not R_PAIRS),
                                skip_group_check=True)

                pend.append(attv)
        for pi in range(R_PAIRS):
            r8 = p_pool.tile([128, 2, N], F8E4, tag="r8", name=f"r8_{hp}_{pi}")
            racc = scr_pool.tile([128, 1], F32, tag="racc", name=f"racc{hp}{pi}")
            for s in range(2):
                kc = P_KC + 2 * pi + s
                lg = [lg_pool.tile([128, NQ], F32, tag="lg",
                                   name=f"lg{hp}_{kc}_{m2}") for m2 in range(2)]
                lg_matmuls(lg, pair, j, kc)
                # r = (z + z^2/2)/SV, z = y*c: t = 1 + z/2 (DVE, psum->
                # sbuf), then (y*(c/SV))*t (one PSUM operand only)
                for m in range(2):
                    tq = scr_pool.tile([128, NQ], BF16, tag="tq",
                                       name=f"tq{hp}_{pi}_{s}_{m}")
                    nc.vector.tensor_scalar(
                        tq, lg[m], float(EXP_SCALE / 2.0), 1.0,
                        op0=ALU.mult, op1=ALU.add)
                    nc.vector.affine_mul_reduce(
                        r8[:, s, m * NQ:(m + 1) * NQ], racc, lg[m], tq,
                        scale=float(EXP_SCALE / SV), bias=0.0)
                for f in pend:
                    f()
                pend = []

            def attv_r(r8=r8, pi=pi):
                for m in range(2):
                    hk = pair[m] // 4
                    for g in range(NQ // 512):
                        nc.tensor.matmul(
                            att_ps[m][:, g * 512:(g + 1) * 512],
                            v8f[:, pi, :, hk, :],
                            r8[:, :, m * NQ + g * 512:m * NQ + (g + 1) * 512],
                            start=False, stop=False, perf_mode=DR,
                            skip_group_check=True)

            pend.append(attv_r)
        for f in pend:
            f()
        if R_PAIRS:
            # rank-1 correction: + Sum_{r-chunks} v (incl. slot count in the
            # ones column) to every query column
            for m in range(2):
                hk = pair[m] // 4
                for g in range(NQ // 512):
                    nc.tensor.matmul(
                        att_ps[m][:, g * 512:(g + 1) * 512],
                        vsum_sb[0:1, hk, :], ones512,
                        start=False, stop=True, skip_group_check=True)
        # fast evict: copy [65, NQ] psum -> sbuf scratch, stash denom row,
        # numerator into oT unscaled; 1/denom once after all pairs.
        for m in range(2):
            h = pair[m]
            scr65 = scr_pool.tile([65, NQ], F32R, tag="scr65",
                                  name=f"scr65_{hp}_{m}")
            nc.vector.tensor_copy(scr65, att_ps[m][0:65, :])
            nc.sync.dma_start(denom_sb[h:h + 1, :],
                              scr65[64:65, :].bitcast(F32))
            if h % 2 == 0:
                nc.vector.tensor_copy(oT_sb[0:64, h // 2, :], scr65[0:64, :])
            else:
                # partition shift 0 -> 64 must go through DMA
                nc.sync.dma_start(oT_sb[64:128, h // 2, :], scr65[0:64, :])

    # denominator: + (N - NSEL) for the never-gathered masked keys, then
    # one batched reciprocal and a per-d-chunk broadcast multiply.
    nc.vector.tensor_scalar(denom_sb, denom_sb, float(N - NSEL), None,
                            op0=ALU.add)
    rec16 = ph2.tile([16, NQ], F32R)
    rec16_f = ph2.tile([16, NQ], F32)
    with nc.allow_low_precision(reason="2e-5 rel err << output tolerance"):
        nc.vector.reciprocal_approx_fast(out=rec16_f, in_=denom_sb)
    nc.vector.tensor_copy(rec16, rec16_f)
    for dd in range(CC):
        for g in range(NQ // 512):
            bps = lg_pool.tile([128, 512], F32, tag="lg", name=f"bps{dd}_{g}")
            nc.tensor.matmul(
                bps, sel8[:, dd, :], rec16[:, g * 512:(g + 1) * 512],
                start=True, stop=True)
            sl = slice(g * 512, (g + 1) * 512)
            nc.vector.tensor_tensor(
                oT_sb[:, dd, sl], oT_sb[:, dd, sl], bps, op=ALU.mult)
    patt_ctx.close()
    # ---------------- phase 3: output projection ----------------
    ph3_ctx = contextlib.ExitStack()
    psum3 = ph3_ctx.enter_context(tc.tile_pool(name="psum3", bufs=4, space="PSUM"))
    out_pool = ph3_ctx.enter_context(tc.tile_pool(name="out_pool", bufs=2))
    for tt in range(NQ // 128):
        out_sb = out_pool.tile([128, C], F32, tag="out_sb", name=f"out_sb{tt}")
        for og in range(C // 512):
            ps = psum3.tile([128, 512], F32, tag="out_ps", name=f"out_ps{tt}_{og}")
            for dd in range(CC):
                nc.tensor.matmul(
                    ps, oT_sb[:, dd, tt * 128:(tt + 1) * 128],
                    wo_sb[:, dd, og * 512:(og + 1) * 512],
                    start=(dd == 0), stop=(dd == CC - 1))
            nc.scalar.copy(out_sb[:, og * 512:(og + 1) * 512], ps)
        nc.sync.dma_start(out_d[tt * 128:(tt + 1) * 128, :], out_sb)
    ph3_ctx.close()
    ph2_ctx.close()


_NC = None


def build_program():
    global _NC
    if _NC is not None:
        return _NC
    from contextlib import ExitStack

    nc = bacc.Bacc("TRN2", target_bir_lowering=False, debug=False, num_devices=8)
    io = {
        "xT": nc.dram_tensor("xT", (C, N), F32, kind="ExternalInput").ap(),
        "wq": nc.dram_tensor("wq", (C, QT_D), BF16, kind="ExternalInput").ap(),
        "wk": nc.dram_tensor("wk", (C, KV_D), BF16, kind="ExternalInput").ap(),
        "wv": nc.dram_tensor("wv", (C, KV_D), BF16, kind="ExternalInput").ap(),
        "rw": nc.dram_tensor("rw", (C, 1), F32, kind="ExternalInput").ap(),
        "wo": nc.dram_tensor("wo", (C, C), F32, kind="ExternalInput").ap(),
        "sel8": nc.dram_tensor("sel8", (16, CC, 128), F32,
                               kind="ExternalInput").ap(),
        "out": nc.dram_tensor("out", (NQ, C), F32, kind="ExternalOutput").ap(),
        "dbg_idx": nc.dram_tensor("dbg_idx", (NSEL,), I16,
                                  kind="ExternalOutput").ap(),
        "dbg_m01": nc.dram_tensor("dbg_m01", (N,), F32,
                                  kind="ExternalOutput").ap(),
    }
    with TileContext(nc) as tc:
        with ExitStack() as ctx:
            _emit(nc, tc, ctx, io)
    nc.compile()
    _NC = nc
    return nc


def _permute_wq(wq):
    """Column-permute wq so QT slot j's 128 cols = heads (ha, ha+4) contig."""
    wq = np.asarray(wq, np.float32).reshape(C, H, DH)
    order = []
    for j in range(H // 2):
        ha = j if j < 4 else j + 4
        order += [ha, ha + 4]
    return np.ascontiguousarray(wq[:, order, :].reshape(C, H * DH))


def make_in_maps(x, router_w, wq, wk, wv, wo):
    wq = _permute_wq(wq)
    in_maps = []
    for core in range(8):
        b, h = core // 2, core % 2
        xT_core = np.ascontiguousarray(
            np.roll(np.asarray(x[b], np.float32).T, -h * NQ, axis=1))
        sel8 = np.zeros((16, CC, 128), np.float32)
        for dd in range(CC):
            for p in range(128):
                sel8[2 * dd + p // 64, dd, p] = 1.0
        in_maps.append({
            "xT": xT_core,
            "sel8": sel8,
            "wq": np.ascontiguousarray(np.asarray(wq, np.float32),
                                       dtype=ml_dtypes.bfloat16),
            "wk": np.ascontiguousarray(np.asarray(wk, np.float32),
                                       dtype=ml_dtypes.bfloat16),
            "wv": np.ascontiguousarray(np.asarray(wv, np.float32),
                                       dtype=ml_dtypes.bfloat16),
            "rw": np.ascontiguousarray(router_w, dtype=np.float32),
            "wo": np.ascontiguousarray(wo, dtype=np.float32),
        })
    return in_maps


def _numpy_fallback(x, router_w, router_b, wq, bq, wk, bk, wv, bv, wo, bo):
    x = np.asarray(x, np.float32)
    gate = 1.0 / (1.0 + np.exp(-(x @ router_w + router_b)))
    xg = x * gate
    scores = gate[..., 0]
    idx = np.argsort(-scores, axis=-1, kind="stable")[:, :KSEL]
    mask = np.zeros((x.shape[0], x.shape[1]), np.float32)
    np.put_along_axis(mask, idx, 1.0, axis=1)
    xg = xg * mask[..., None]
    q = (xg @ wq + bq).reshape(B, N, H, DH)
    kk = np.repeat((xg @ wk + bk).reshape(B, N, HKV, DH), H // HKV, axis=2)
    v = np.repeat((xg @ wv + bv).reshape(B, N, HKV, DH), H // HKV, axis=2)
    att = np.einsum("bqhd,bkhd->bhqk", q, kk) / np.float32(np.sqrt(DH))
    att = att - att.max(-1, keepdims=True)
    att = np.exp(att)
    att = att / att.sum(-1, keepdims=True)
    o = np.einsum("bhqk,bkhd->bqhd", att, v).reshape(B, N, C)
    return (o @ wo + bo).astype(np.float32)


def kernel(x, router_w, router_b, wq, bq, wk, bk, wv, bv, wo, bo):
    x = np.asarray(x)
    biases = [router_b, bq, bk, bv, bo]
    if any(float(np.abs(np.asarray(t)).max()) != 0.0 for t in biases):
        # The device program folds away the (identically zero) biases; fall
        # back to an exact host implementation if that assumption breaks.
        return _numpy_fallback(x, router_w, router_b, wq, bq, wk, bk, wv, bv,
                               wo, bo)

    from concourse import bass_utils

    nc = build_program()
    in_maps = make_in_maps(x, router_w, wq, wk, wv, wo)
    res = bass_utils.run_bass_kernel_spmd(nc, in_maps, core_ids=list(range(8)))
    out = np.empty((B, N, C), np.float32)
    for core in range(8):
        b, h = core // 2, core % 2
        out[b, h * NQ:(h + 1) * NQ, :] = res.results[core]["out"]
    return out



# revision 29
# speedup vs baseline: 1.3407x; 1.0128x over previous
"""Trainium2 Bass kernel for MIGAttention (topk token masking + GQA attention).

Shapes (hardcoded): B=4, N=2048, C=1024, H=16 heads, HKV=4 kv-heads, DH=64,
keep-ratio 0.7 -> k = 1433 selected tokens per batch row.

Sharding: 8 cores = (batch b in 0..3) x (query-half h in 0..1).  Each core
receives x[b].T with token columns rolled by h*1024 so that its own query
half always occupies columns 0..1023 -> a single SPMD program for all cores.

Key structure (v2): the top-k mask selects 1433 of 2048 tokens; masked tokens
have zero K/V rows.  The kernel compacts the selected tokens on device
(prefix-sum + indirect-DMA scatter/gather) and runs attention over only
NSEL=1536 gathered key slots (12 chunks instead of 16).  Junk tail slots
(gathered masked tokens) have zero K/V so their logits are exactly 0 and
p=exp(0)=1, standing in one-for-one for masked tokens in the softmax
denominator; the remaining 2048-1536=512 masked keys are a static +512
constant added to the denominator.

QK_FP8: Q/K are quantized to fp8e4 at the projection evict and the logits
matmuls run in DoubleRow perf mode (2x).  R_PAIRS: the last 2*R_PAIRS key
chunks compute r = expm1(y) ~ y + y^2/2 on the Vector engine (one fused
affine_mul op straight from PSUM) in fp8, and their att@V uses fp8 DoubleRow
with the exact Sum(v) rank-1 correction folded into the same PSUM
accumulation.  This keeps all fp8 error on the small residual r (rms ~0.1)
instead of p (~1.0).
"""

import contextlib
import sys

import ml_dtypes
import numpy as np

if "/opt/trn_rl_repo" not in sys.path:
    sys.path.insert(0, "/opt/trn_rl_repo")

import concourse.bass as bass  # noqa: F401
import concourse.bass_isa as bass_isa
import concourse.mybir as mybir
from concourse import bacc
from concourse.tile import TileContext

F32 = mybir.dt.float32
F32R = mybir.dt.float32r
BF16 = mybir.dt.bfloat16
F8E4 = mybir.dt.float8e4
I32 = mybir.dt.int32
I16 = mybir.dt.int16
AF = mybir.ActivationFunctionType
ALU = mybir.AluOpType
DR = mybir.MatmulPerfMode.DoubleRow

B, N, C = 4, 2048, 1024
H, HKV, DH = 16, 4, 64
NQ = N // 2          # queries per core
KSEL = 1433          # max(1, int(N * 0.7))
CC = C // 128        # contraction chunks (8)
KC = N // 128        # dense token chunks (16)
KCG = 12             # gathered key chunks (1536 slots >= KSEL)
NSEL = KCG * 128     # 1536
QT_D = H * DH        # 1024
KV_D = HKV * DH      # 256
N_ROUNDS = 4         # topk threshold refinement rounds (8/128^4 ~ 3e-8 << min topk gap 3.5e-6)
LO0, W0 = -4.0, 8.0  # initial logit search interval (logit std ~0.65)

# ---- tuning flags ----
QK_FP8 = True        # fp8 Q/K + DoubleRow logits matmuls
R_PAIRS = 0          # of the 12 chunks, last 2*R_PAIRS use the DVE r-route
P_KC = KCG - 2 * R_PAIRS
SQK = 32.0           # fp8 quantization scale for q/k
SV = 4.0             # fp8 v scale for the r-route (r is divided by SV)
EXP_SCALE = 1.0 / (np.sqrt(DH) * (SQK * SQK if QK_FP8 else 1.0))
KDT = F8E4 if QK_FP8 else BF16
GSUB, NSUB = 384, NSEL // 384  # SWDGE sub-gather split


def _emit(nc, tc, ctx, io):
    xT, wq, wk, wv, rw, wo, out_d = (
        io["xT"], io["wq"], io["wk"], io["wv"], io["rw"], io["wo"], io["out"])

    # ---------------- long-lived pools ----------------
    const = ctx.enter_context(tc.tile_pool(name="const", bufs=1))
    small = ctx.enter_context(tc.tile_pool(name="small", bufs=1))
    big = ctx.enter_context(tc.tile_pool(name="big", bufs=1))
    dram = ctx.enter_context(tc.tile_pool(name="dram", bufs=1, space="DRAM"))

    # nesting (LIFO): px > pm > {pa, pcs, pg > {pq, pkv}}
    px_ctx = contextlib.ExitStack()   # xT (alive through projections+gather)
    pm_ctx = contextlib.ExitStack()   # m_rep/m_v/compaction sbuf scratch
    pa_ctx = contextlib.ExitStack()   # router/refinement scratch
    pg_ctx = contextlib.ExitStack()   # proj staging + transpose psum
    pq_ctx = contextlib.ExitStack()   # wq
    pkv_ctx = contextlib.ExitStack()  # wk, wv
    px = px_ctx.enter_context(tc.tile_pool(name="px", bufs=1))
    pm = pm_ctx.enter_context(tc.tile_pool(name="pm", bufs=1))
    pa = pa_ctx.enter_context(tc.tile_pool(name="pa", bufs=1))
    psum_r = pa_ctx.enter_context(tc.tile_pool(name="psum_r", bufs=1, space="PSUM"))

    # ---------------- constants ----------------
    ones_row = const.tile([1, 128], F32)
    nc.vector.memset(ones_row, 1.0)
    ones512 = const.tile([1, 512], BF16)
    nc.vector.memset(ones512, 1.0)
    onescol_bf = const.tile([128, 1], BF16)
    nc.vector.memset(onescol_bf, 1.0)
    iota128_i = const.tile([128, 1], I32)
    nc.gpsimd.iota(iota128_i, pattern=[[0, 1]], base=1, channel_multiplier=1)
    iota128 = const.tile([128, 1], F32)
    nc.vector.tensor_copy(iota128, iota128_i)
    sel8 = const.tile([16, CC, 128], F32R)
    nc.sync.dma_start(sel8, io["sel8"].bitcast(F32R))

    # ---------------- DRAM scratch ----------------
    m_dram = dram.tile([N], F32)
    idx16_dram = dram.tile([NSEL], I16)
    k_dram = dram.tile([N, KV_D], BF16)
    v_dram = dram.tile([N, KV_D], BF16)

    # ---------------- router: logits = x @ rw, exact fp32 ----------------
    rw_sb = pa.tile([128, CC], F32)
    for cc in range(CC):
        sl = slice(cc * 128, (cc + 1) * 128)
        nc.sync.dma_start(rw_sb[:, cc:cc + 1], rw[sl, :])
    # Single fp32 x load feeds both the exact-fp32 PE router matmuls (the
    # instruction structure must match the reference summation closely: batch
    # 0's topk threshold gap is 3.5e-6, so any other reduction order flips a
    # near-tie - measured both for f32r and for a DVE-partials variant) and,
    # via engine casts, the bf16 working copy the projections use.
    logits_sb = pa.tile([1, N], F32)
    xr_pool = pa_ctx.enter_context(tc.tile_pool(name="xr_pool", bufs=3))
    xb = px.tile([128, CC, N], BF16)
    rps = [psum_r.tile([1, 512], F32, tag=f"router_ps{g}", name=f"router_ps{g}")
           for g in range(4)]
    for cc in range(CC):
        xr = xr_pool.tile([128, N], F32, tag="xr", name=f"xr{cc}")
        eng = nc.sync if cc % 2 == 0 else nc.scalar
        eng.dma_start(xr, xT[cc * 128:(cc + 1) * 128, :])
        for g in range(4):
            nc.tensor.matmul(
                rps[g], rw_sb[:, cc:cc + 1], xr[:, g * 512:(g + 1) * 512],
                start=(cc == 0), stop=(cc == CC - 1))
        if cc % 2 == 0:
            nc.scalar.copy(xb[:, cc, :], xr)
        else:
            nc.vector.tensor_copy(xb[:, cc, :], xr)
    for g in range(4):
        nc.vector.tensor_copy(logits_sb[:, g * 512:(g + 1) * 512], rps[g])

    # replicate logits across all 128 partitions (K=1 matmul broadcast)
    lrep = pa.tile([128, N], F32)
    for g in range(4):
        ps = psum_r.tile([128, 512], F32, tag="bcast_ps")
        nc.tensor.matmul(ps, ones_row, logits_sb[:, g * 512:(g + 1) * 512],
                         start=True, stop=True)
        nc.vector.tensor_copy(lrep[:, g * 512:(g + 1) * 512], ps)

    # ---------------- topk threshold refinement ----------------
    # invariant: v* (the KSEL-th largest logit) is in (lo, lo + w]
    lo = small.tile([128, 1], F32)
    nc.vector.memset(lo, LO0)
    neg_edges = small.tile([128, 1], F32)
    acc = small.tile([128, 1], F32)
    sel = small.tile([128, 1], F32)
    ssum = small.tile([128, 1], F32)
    sign_scr = pa.tile([128, N], BF16)  # Sign output is never read
    thr_acc = float(2 * KSEL - N)  # acc = #gt - #lt ; acc>=thr <=> #gt>=KSEL
    for r in range(N_ROUNDS):
        wstep = W0 / (128.0 ** (r + 1))
        nc.vector.scalar_tensor_tensor(
            neg_edges, iota128, -wstep, lo, op0=ALU.mult, op1=ALU.subtract)
        nc.scalar.activation(sign_scr, lrep, AF.Sign, bias=neg_edges,
                             scale=1.0, accum_out=acc)
        nc.vector.tensor_single_scalar(sel, acc, thr_acc, op=ALU.is_ge)
        nc.gpsimd.partition_all_reduce(ssum, sel, channels=128,
                                       reduce_op=bass_isa.ReduceOp.add)
        nc.vector.scalar_tensor_tensor(
            lo, ssum, wstep, lo, op0=ALU.mult, op1=ALU.add)

    # m = (logit > lo) * sigmoid(logit)   per token, replicated on partitions
    grep = pa.tile([128, N], F32)
    nc.scalar.activation(grep, lrep, AF.Sigmoid)
    m_rep = pm.tile([128, N], F32)
    nc.vector.scalar_tensor_tensor(
        m_rep, lrep, lo, grep, op0=ALU.is_gt, op1=ALU.mult)

    # token-major copies: m_v[p, i] = m[i*128 + p]
    nc.scalar.dma_start(m_dram, m_rep[0:1, :])
    pa_ctx.close()
    pc = pm
    m_v = pc.tile([128, KC], F32)
    nc.scalar.dma_start(m_v, m_dram.rearrange("(i p) -> p i", p=128))
    if QK_FP8:
        msc_v = pc.tile([128, KC], F32)
        nc.vector.tensor_scalar(msc_v, m_v, SQK, None, op0=ALU.mult)
    else:
        msc_v = m_v

    # ---------------- compaction: selected-token index list ----------------
    # Single-partition row pipeline (pure DVE + one gpsimd scatter; nothing on
    # the PE queue, no DRAM roundtrips): csel[t] = #selected in tokens [0, t],
    # pos[t] = selected ? csel-1 : count + t - csel, clamped to < NSEL.
    m01row = pc.tile([1, N], F32)
    nc.vector.tensor_scalar(m01row, lrep[0:1, :], lo[0:1, :], None,
                            op0=ALU.is_gt)
    csel = pc.tile([1, N], F32)
    nc.vector.tensor_tensor_scan(csel, m01row, m01row, 0.0,
                                 op0=ALU.add, op1=ALU.bypass)
    r_i = pc.tile([1, N], I32)
    nc.gpsimd.iota(r_i, pattern=[[1, N]], base=0, channel_multiplier=0)
    pos = pc.tile([1, N], F32)
    nc.vector.tensor_copy(pos, r_i)           # pos <- t
    nc.vector.tensor_tensor(pos, pos, csel, op=ALU.subtract)
    nc.vector.tensor_scalar(pos, pos, csel[:, N - 1:N], None, op0=ALU.add)
    scr = pc.tile([1, N], F32)
    nc.vector.tensor_scalar(scr, csel, -1.0, None, op0=ALU.add)  # csel-1
    nc.vector.tensor_copy(r_i, m01row)        # int mask
    nc.vector.copy_predicated(pos, r_i, scr)
    # clamp: tokens landing beyond the NSEL gathered slots -> -1 (dropped)
    nc.vector.tensor_single_scalar(scr, pos, float(NSEL), op=ALU.is_lt)
    nc.vector.scalar_tensor_tensor(pos, pos, 1.0, scr,
                                   op0=ALU.add, op1=ALU.mult)
    # single-partition compaction: sc_out[0, pos[t]] = t for pos >= 0
    sc_idx = pc.tile([16, N], I16)
    nc.vector.memset(sc_idx, -1)
    nc.vector.tensor_scalar(sc_idx[0:1, :], pos, -1.0, None, op0=ALU.add)
    sc_data = pc.tile([16, N], I16)
    nc.gpsimd.iota(sc_data, pattern=[[1, N]], base=0, channel_multiplier=0)
    sc_out = pc.tile([16, NSEL], I16)
    nc.gpsimd.local_scatter(sc_out, sc_data, sc_idx, channels=16,
                            num_elems=NSEL, num_idxs=N)
    nc.scalar.dma_start(idx16_dram, sc_out[0:1, :])
    nc.scalar.dma_start(io["dbg_idx"], sc_out[0:1, :])
    nc.scalar.dma_start(io["dbg_m01"], m01row)
    # wrapped gather-index layout: idx j -> [j % 16, j // 16], replicated
    idx16_sb = big.tile([128, NSEL // 16], I16)
    for k in range(8):
        nc.scalar.dma_start(idx16_sb[16 * k:16 * (k + 1), :],
                          idx16_dram.rearrange("(s p) -> p s", p=16))
    stg_pool = pg_ctx.enter_context(tc.tile_pool(name="stg_pool", bufs=3))
    psum1 = pg_ctx.enter_context(tc.tile_pool(name="psum1", bufs=5, space="PSUM"))

    # ---------------- K projection, token-major -> DRAM rows ----------------
    pkv = pkv_ctx.enter_context(tc.tile_pool(name="pkv", bufs=1))
    wk_sb = pkv.tile([128, CC, KV_D], BF16)
    wv_sb = pkv.tile([128, CC, KV_D], BF16)
    for cc in range(CC):
        sl = slice(cc * 128, (cc + 1) * 128)
        nc.sync.dma_start(wk_sb[:, cc, :], wk[sl, :])
        nc.sync.dma_start(wv_sb[:, cc, :], wv[sl, :])
    # ---------------- V projection + gathers ----------------
    for i2 in range(KC // 2):
        vps = psum1.tile([128, 512], F32, tag="proj_ps", name=f"v_ps{i2}")
        for half in range(2):
            i = 2 * i2 + half
            ts_ = slice(i * 128, (i + 1) * 128)
            vp = vps[:, half * KV_D:(half + 1) * KV_D]
            for cc in range(CC):
                nc.tensor.matmul(vp, xb[:, cc, ts_], wv_sb[:, cc, :],
                                 start=(cc == 0), stop=(cc == CC - 1))
        vstage = stg_pool.tile([128, 2, KV_D], BF16, tag="vstage",
                               name=f"vstage{i2}")
        nc.scalar.activation(vstage[:, 0, :], vps[:, 0:KV_D], AF.Identity,
                             scale=m_v[:, 2 * i2:2 * i2 + 1])
        nc.scalar.activation(vstage[:, 1, :], vps[:, KV_D:2 * KV_D],
                             AF.Identity, scale=m_v[:, 2 * i2 + 1:2 * i2 + 2])
        nc.sync.dma_start(
            v_dram[2 * i2 * 128:(2 * i2 + 2) * 128, :]
            .rearrange("(c t) d -> t c d", t=128), vstage)
    v8g = stg_pool.tile([128, KCG, KV_D], BF16, tag="v8g")
    v8_sb = big.tile([128, KCG, HKV, 128], BF16)
    nc.vector.memset(v8_sb.bitcast(I32), 0)
    nc.vector.memset(v8_sb[:, :, :, 64:65], 1.0)
    for g in range(NSUB):
        ic = idx16_sb[:, g * (GSUB // 16):(g + 1) * (GSUB // 16)]
        cs = slice(g * (GSUB // 128), (g + 1) * (GSUB // 128))
        nc.gpsimd.dma_gather(v8g[:, cs, :], v_dram[:, :], ic,
                             GSUB, GSUB, elem_size=KV_D, transpose=False)
        nc.vector.tensor_copy(
            v8_sb[:, cs, :, 0:64],
            v8g[:, cs, :].rearrange("p c (h e) -> p c h e", e=64))

    # ---------------- K projection, token-major -> DRAM rows ----------------
    for i2 in range(KC // 2):
        kps = psum1.tile([128, 512], F32, tag="proj_ps", name=f"k_ps{i2}")
        for half in range(2):
            i = 2 * i2 + half
            ts_ = slice(i * 128, (i + 1) * 128)
            kp = kps[:, half * KV_D:(half + 1) * KV_D]
            for cc in range(CC):
                nc.tensor.matmul(kp, xb[:, cc, ts_], wk_sb[:, cc, :],
                                 start=(cc == 0), stop=(cc == CC - 1))
        kstage = stg_pool.tile([128, 2, KV_D], BF16, tag="kstage",
                               name=f"kstage{i2}")
        nc.scalar.activation(kstage[:, 0, :], kps[:, 0:KV_D], AF.Identity,
                             scale=msc_v[:, 2 * i2:2 * i2 + 1])
        nc.scalar.activation(kstage[:, 1, :], kps[:, KV_D:2 * KV_D],
                             AF.Identity,
                             scale=msc_v[:, 2 * i2 + 1:2 * i2 + 2])
        nc.sync.dma_start(
            k_dram[2 * i2 * 128:(2 * i2 + 2) * 128, :]
            .rearrange("(c t) d -> t c d", t=128), kstage)

    # ---------------- K gathers (SWDGE), pipelined into kt_z ----------------
    # dma_gather transpose mode: out[d % 128, d // 128, slot] = k[idx[slot], d]
    # -> directly the [dims-on-partitions, token-free] layout the logits
    # matmuls need.  kt_z: two parity copies with the other 64-partition half
    # zeroed, so the logits matmul contracts a full K=128 (partner q rows hit
    # zeros) and the PE HAM activity monitor stays warm.  QK_FP8 adds a
    # zeroed second DoubleRow subtile.  Sub-gathers stay well under the SWDGE
    # descriptor-ring capacity (16KB carveout = 1024 descriptors).
    kt_all = stg_pool.tile([128, NSUB, 2, GSUB], BF16, tag="kt_all")
    if QK_FP8:
        kt_z = big.tile([128, 2, 2, 2, NSEL], F8E4)   # [p, par, sub, j, t]
        nc.vector.memset(kt_z[64:128, 0, :, :, :].bitcast(I32), 0)
        nc.vector.memset(kt_z[0:64, 1, :, :, :].bitcast(I32), 0)
        nc.vector.memset(kt_z[:, :, 1, :, :].bitcast(I32), 0)
    else:
        kt_z = big.tile([128, 2, 2, NSEL], BF16)      # [p, par, j, t]
        nc.vector.memset(kt_z[64:128, 0, :, :].bitcast(I32), 0)
        nc.vector.memset(kt_z[0:64, 1, :, :].bitcast(I32), 0)
    for g in range(NSUB):
        ic = idx16_sb[:, g * (GSUB // 16):(g + 1) * (GSUB // 16)]
        nc.gpsimd.dma_gather(kt_all[:, g, :, :], k_dram[:, :], ic,
                             GSUB, GSUB, elem_size=KV_D, transpose=True)
        gs = slice(g * GSUB, (g + 1) * GSUB)
        for j in range(2):
            if QK_FP8:
                nc.vector.tensor_copy(kt_z[0:64, 0, 0, j, gs],
                                      kt_all[0:64, g, j, :])
                nc.vector.tensor_copy(kt_z[64:128, 1, 0, j, gs],
                                      kt_all[64:128, g, j, :])
            else:
                nc.vector.tensor_copy(kt_z[0:64, 0, j, gs],
                                      kt_all[0:64, g, j, :])
                nc.vector.tensor_copy(kt_z[64:128, 1, j, gs],
                                      kt_all[64:128, g, j, :])

    pkv_ctx.close()
    # ---------------- QT projection (overlaps the gather chain) ------------
    # Slot layout is permuted so each q-head lands on the same partition range
    # as its GQA kv-head in KT: head h -> slot (h%4)+4*(h//8), partition base
    # ((h//4)%2)*64.  Slot j holds heads (ha, ha+4), ha = j if j<4 else j+4.
    pq = pq_ctx.enter_context(tc.tile_pool(name="pq", bufs=1))
    wq_sb = pq.tile([128, CC, QT_D], BF16)
    for cc in range(CC):
        nc.sync.dma_start(wq_sb[:, cc, :], wq[cc * 128:(cc + 1) * 128, :])
    if QK_FP8:
        qt_sb = big.tile([128, H // 2, 2, NQ], F8E4)
        nc.vector.memset(qt_sb[:, :, 1, :].bitcast(I32), 0)
    else:
        qt_sb = big.tile([128, H // 2, NQ], BF16)
    for j in range(H // 2):
        for g in range(NQ // 512):
            ps = psum1.tile([128, 512], F32, tag="proj_ps",
                            name=f"q_ps{j}_{g}")
            qs = slice(g * 512, (g + 1) * 512)
            for cc in range(CC):
                nc.tensor.matmul(
                    ps, wq_sb[:, cc, j * 128:(j + 1) * 128],
                    xb[:, cc, qs],
                    start=(cc == 0), stop=(cc == CC - 1))
            if QK_FP8:
                nc.vector.scalar_tensor_tensor(
                    qt_sb[:, j, 0, qs], ps, SQK, m_rep[:, qs],
                    op0=ALU.mult, op1=ALU.mult)
            else:
                nc.vector.tensor_tensor(qt_sb[:, j, qs], ps, m_rep[:, qs],
                                        op=ALU.mult)
    pq_ctx.close()

    # r-route prep: fp8 V copies + exact Sum(v) over the r-chunks
    if R_PAIRS:
        psum_t = pg_ctx.enter_context(
            tc.tile_pool(name="psum_t", bufs=1, space="PSUM"))
        v8f = big.tile([128, R_PAIRS, 2, HKV, 128], F8E4)
        vsum_sb = big.tile([1, HKV, 128], BF16)
        vs_ps = psum_t.tile([1, 512], F32, tag="vs", name="vs_ps")
        for pi in range(R_PAIRS):
            for s in range(2):
                rc = P_KC + 2 * pi + s
                nc.vector.tensor_scalar(
                    v8f[:, pi, s, :, :], v8_sb[:, rc, :, :], SV, None,
                    op0=ALU.mult)
                nc.tensor.matmul(vs_ps, onescol_bf, v8_sb[:, rc, :, :],
                                 start=(pi == 0 and s == 0),
                                 stop=(pi == R_PAIRS - 1 and s == 1))
        nc.vector.tensor_copy(vsum_sb, vs_ps)
    pg_ctx.close()
    pm_ctx.close()
    px_ctx.close()  # free xT + phase-1 PSUM

    # ---------------- phase 2: attention ----------------
    ph2_ctx = contextlib.ExitStack()
    ph2 = ph2_ctx.enter_context(tc.tile_pool(name="ph2", bufs=1))
    wo_sb = ph2.tile([128, CC, C], F32R)
    for cc in range(CC):
        nc.sync.dma_start(wo_sb[:, cc, :],
                          wo[cc * 128:(cc + 1) * 128, :].bitcast(F32R))

    patt_ctx = contextlib.ExitStack()
    scr_pool = patt_ctx.enter_context(tc.tile_pool(name="scr_pool", bufs=2))
    p_pool = patt_ctx.enter_context(tc.tile_pool(name="p_pool", bufs=2))
    lg_pool = patt_ctx.enter_context(
        tc.tile_pool(name="lg_pool", bufs=2, space="PSUM"))
    att_pool = patt_ctx.enter_context(
        tc.tile_pool(name="att_pool", bufs=1, space="PSUM"))
    oT_sb = ph2.tile([128, CC, NQ], F32R)
    denom_sb = ph2.tile([16, NQ], F32)

    def lg_matmuls(lg, pair, j, kc):
        """logits for both heads of the pair into lg[0], lg[1]."""
        for m in range(2):
            jq = (pair[m] % 4) + 4 * (pair[m] // 8)
            for g in range(NQ // 512):
                gs = slice(g * 512, (g + 1) * 512)
                if QK_FP8:
                    nc.tensor.matmul(
                        lg[m][:, gs],
                        kt_z[:, m, :, j, kc * 128:(kc + 1) * 128],
                        qt_sb[:, jq, :, gs],
                        start=True, stop=True, perf_mode=DR,
                        skip_group_check=True)
                else:
                    nc.tensor.matmul(
                        lg[m][:, gs],
                        kt_z[:, m, j, kc * 128:(kc + 1) * 128],
                        qt_sb[:, jq, gs],
                        start=True, stop=True)

    pair_heads = [(ha, ha + 4) for ha in (0, 1, 2, 3, 8, 9, 10, 11)]
    for hp, pair in enumerate(pair_heads):
        j = pair[0] // 8
        att_ps = [att_pool.tile([128, NQ], F32, tag=f"att{m}", name=f"att{hp}_{m}")
                  for m in range(2)]
        pend = []  # pipelined attv matmuls: emitted one chunk behind
        for quarter in range(P_KC // 2):
            p_t = p_pool.tile([128, 2, N], BF16, tag="p_t",
                              name=f"p_{hp}_{quarter}")
            for kci in range(2):
                kc = quarter * 2 + kci
                lg = [lg_pool.tile([128, NQ], F32, tag="lg",
                                   name=f"lg{hp}_{kc}_{m2}") for m2 in range(2)]
                lg_matmuls(lg, pair, j, kc)
                for m in range(2):
                    nc.scalar.activation(
                        p_t[:, kci, m * NQ:(m + 1) * NQ], lg[m], AF.Exp,
                        scale=EXP_SCALE)
                for f in pend:
                    f()
                pend = []

                def attv(p_t=p_t, kci=kci, kc=kc):
                    for m in range(2):
                        hk = pair[m] // 4
                        for g in range(NQ // 512):
                            nc.tensor.matmul(
                                att_ps[m][:, g * 512:(g + 1) * 512],
                                v8_sb[:, kc, hk, :],
                                p_t[:, kci,
                                    m * NQ + g * 512:m * NQ + (g + 1) * 512],
                                start=(kc == 0),
                                stop=(kc == KCG - 1 and not R_PAIRS),
                                skip_group_check=True)

                pend.append(attv)
        for pi in range(R_PAIRS):
            r8 = p_pool.tile([128, 2, N], F8E4, tag="r8", name=f"r8_{hp}_{pi}")
            racc = scr_pool.tile([128, 1], F32, tag="racc", name=f"racc{hp}{pi}")
            for s in range(2):
                kc = P_KC + 2 * pi + s
                lg = [lg_pool.tile([128, NQ], F32, tag="lg",
                                   name=f"lg{hp}_{kc}_{m2}") for m2 in range(2)]
                lg_matmuls(lg, pair, j, kc)
                # r = (z + z^2/2)/SV, z = y*c: t = 1 + z/2 (DVE, psum->
                # sbuf), then (y*(c/SV))*t (one PSUM operand only)
                for m in range(2):
                    tq = scr_pool.tile([128, NQ], BF16, tag="tq",
                                       name=f"tq{hp}_{pi}_{s}_{m}")
                    nc.vector.tensor_scalar(
                        tq, lg[m], float(EXP_SCALE / 2.0), 1.0,
                        op0=ALU.mult, op1=ALU.add)
                    nc.vector.affine_mul_reduce(
                        r8[:, s, m * NQ:(m + 1) * NQ], racc, lg[m], tq,
                        scale=float(EXP_SCALE / SV), bias=0.0)
                for f in pend:
                    f()
                pend = []

            def attv_r(r8=r8, pi=pi):
                for m in range(2):
                    hk = pair[m] // 4
                    for g in range(NQ // 512):
                        nc.tensor.matmul(
                            att_ps[m][:, g * 512:(g + 1) * 512],
                            v8f[:, pi, :, hk, :],
                            r8[:, :, m * NQ + g * 512:m * NQ + (g + 1) * 512],
                            start=False, stop=False, perf_mode=DR,
                            skip_group_check=True)

            pend.append(attv_r)
        for f in pend:
            f()
        if R_PAIRS:
            # rank-1 correction: + Sum_{r-chunks} v (incl. slot count in the
            # ones column) to every query column
            for m in range(2):
                hk = pair[m] // 4
                for g in range(NQ // 512):
                    nc.tensor.matmul(
                        att_ps[m][:, g * 512:(g + 1) * 512],
                        vsum_sb[0:1, hk, :], ones512,
                        start=False, stop=True, skip_group_check=True)
        # fast evict: copy [65, NQ] psum -> sbuf scratch, stash denom row,
        # numerator into oT unscaled; 1/denom once after all pairs.
        for m in range(2):
            h = pair[m]
            scr65 = scr_pool.tile([65, NQ], F32R, tag="scr65",
                                  name=f"scr65_{hp}_{m}")
            nc.vector.tensor_copy(scr65, att_ps[m][0:65, :])
            nc.sync.dma_start(denom_sb[h:h + 1, :],
                              scr65[64:65, :].bitcast(F32))
            if h % 2 == 0:
                nc.vector.tensor_copy(oT_sb[0:64, h // 2, :], scr65[0:64, :])
            else:
                # partition shift 0 -> 64 must go through DMA
                nc.sync.dma_start(oT_sb[64:128, h // 2, :], scr65[0:64, :])

    # denominator: + (N - NSEL) for the never-gathered masked keys, then
    # one batched reciprocal and a per-d-chunk broadcast multiply.
    nc.vector.tensor_scalar(denom_sb, denom_sb, float(N - NSEL), None,
                            op0=ALU.add)
    rec16 = ph2.tile([16, NQ], F32R)
    rec16_f = ph2.tile([16, NQ], F32)
    with nc.allow_low_precision(reason="2e-5 rel err << output tolerance"):
        nc.vector.reciprocal_approx_fast(out=rec16_f, in_=denom_sb)
    nc.vector.tensor_copy(rec16, rec16_f)
    for dd in range(CC):
        for g in range(NQ // 512):
            bps = lg_pool.tile([128, 512], F32, tag="lg", name=f"bps{dd}_{g}")
            nc.tensor.matmul(
                bps, sel8[:, dd, :], rec16[:, g * 512:(g + 1) * 512],
                start=True, stop=True)
            sl = slice(g * 512, (g + 1) * 512)
            nc.vector.tensor_tensor(
                oT_sb[:, dd, sl], oT_sb[:, dd, sl], bps, op=ALU.mult)
    patt_ctx.close()
    # ---------------- phase 3: output projection ----------------
    ph3_ctx = contextlib.ExitStack()
    psum3 = ph3_ctx.enter_context(tc.tile_pool(name="psum3", bufs=4, space="PSUM"))
    out_pool = ph3_ctx.enter_context(tc.tile_pool(name="out_pool", bufs=2))
    for tt in range(NQ // 128):
        out_sb = out_pool.tile([128, C], F32, tag="out_sb", name=f"out_sb{tt}")
        for og in range(C // 512):
            ps = psum3.tile([128, 512], F32, tag="out_ps", name=f"out_ps{tt}_{og}")
            for dd in range(CC):
                nc.tensor.matmul(
                    ps, oT_sb[:, dd, tt * 128:(tt + 1) * 128],
                    wo_sb[:, dd, og * 512:(og + 1) * 512],
                    start=(dd == 0), stop=(dd == CC - 1))
            nc.scalar.copy(out_sb[:, og * 512:(og + 1) * 512], ps)
        nc.sync.dma_start(out_d[tt * 128:(tt + 1) * 128, :], out_sb)
    ph3_ctx.close()
    ph2_ctx.close()


_NC = None


def build_program():
    global _NC
    if _NC is not None:
        return _NC
    from contextlib import ExitStack

    nc = bacc.Bacc("TRN2", target_bir_lowering=False, debug=False, num_devices=8)
    io = {
        "xT": nc.dram_tensor("xT", (C, N), F32, kind="ExternalInput").ap(),
        "wq": nc.dram_tensor("wq", (C, QT_D), BF16, kind="ExternalInput").ap(),
        "wk": nc.dram_tensor("wk", (C, KV_D), BF16, kind="ExternalInput").ap(),
        "wv": nc.dram_tensor("wv", (C, KV_D), BF16, kind="ExternalInput").ap(),
        "rw": nc.dram_tensor("rw", (C, 1), F32, kind="ExternalInput").ap(),
        "wo": nc.dram_tensor("wo", (C, C), F32, kind="ExternalInput").ap(),
        "sel8": nc.dram_tensor("sel8", (16, CC, 128), F32,
                               kind="ExternalInput").ap(),
        "out": nc.dram_tensor("out", (NQ, C), F32, kind="ExternalOutput").ap(),
        "dbg_idx": nc.dram_tensor("dbg_idx", (NSEL,), I16,
                                  kind="ExternalOutput").ap(),
        "dbg_m01": nc.dram_tensor("dbg_m01", (N,), F32,
                                  kind="ExternalOutput").ap(),
    }
    with TileContext(nc) as tc:
        with ExitStack() as ctx:
            _emit(nc, tc, ctx, io)
    nc.compile()
    _NC = nc
    return nc


def _permute_wq(wq):
    """Column-permute wq so QT slot j's 128 cols = heads (ha, ha+4) contig."""
    wq = np.asarray(wq, np.float32).reshape(C, H, DH)
    order = []
    for j in range(H // 2):
        ha = j if j < 4 else j + 4
        order += [ha, ha + 4]
    return np.ascontiguousarray(wq[:, order, :].reshape(C, H * DH))


def make_in_maps(x, router_w, wq, wk, wv, wo):
    wq = _permute_wq(wq)
    in_maps = []
    for core in range(8):
        b, h = core // 2, core % 2
        xT_core = np.ascontiguousarray(
            np.roll(np.asarray(x[b], np.float32).T, -h * NQ, axis=1))
        sel8 = np.zeros((16, CC, 128), np.float32)
        for dd in range(CC):
            for p in range(128):
                sel8[2 * dd + p // 64, dd, p] = 1.0
        in_maps.append({
            "xT": xT_core,
            "sel8": sel8,
            "wq": np.ascontiguousarray(np.asarray(wq, np.float32),
                                       dtype=ml_dtypes.bfloat16),
            "wk": np.ascontiguousarray(np.asarray(wk, np.float32),
                                       dtype=ml_dtypes.bfloat16),
            "wv": np.ascontiguousarray(np.asarray(wv, np.float32),
                                       dtype=ml_dtypes.bfloat16),
            "rw": np.ascontiguousarray(router_w, dtype=np.float32),
            "wo": np.ascontiguousarray(wo, dtype=np.float32),
        })
    return in_maps


def _numpy_fallback(x, router_w, router_b, wq, bq, wk, bk, wv, bv, wo, bo):
    x = np.asarray(x, np.float32)
    gate = 1.0 / (1.0 + np.exp(-(x @ router_w + router_b)))
    xg = x * gate
    scores = gate[..., 0]
    idx = np.argsort(-scores, axis=-1, kind="stable")[:, :KSEL]
    mask = np.zeros((x.shape[0], x.shape[1]), np.float32)
    np.put_along_axis(mask, idx, 1.0, axis=1)
    xg = xg * mask[..., None]
    q = (xg @ wq + bq).reshape(B, N, H, DH)
    kk = np.repeat((xg @ wk + bk).reshape(B, N, HKV, DH), H // HKV, axis=2)
    v = np.repeat((xg @ wv + bv).reshape(B, N, HKV, DH), H // HKV, axis=2)
    att = np.einsum("bqhd,bkhd->bhqk", q, kk) / np.float32(np.sqrt(DH))
    att = att - att.max(-1, keepdims=True)
    att = np.exp(att)
    att = att / att.sum(-1, keepdims=True)
    o = np.einsum("bhqk,bkhd->bqhd", att, v).reshape(B, N, C)
    return (o @ wo + bo).astype(np.float32)


def kernel(x, router_w, router_b, wq, bq, wk, bk, wv, bv, wo, bo):
    x = np.asarray(x)
    biases = [router_b, bq, bk, bv, bo]
    if any(float(np.abs(np.asarray(t)).max()) != 0.0 for t in biases):
        # The device program folds away the (identically zero) biases; fall
        # back to an exact host implementation if that assumption breaks.
        return _numpy_fallback(x, router_w, router_b, wq, bq, wk, bk, wv, bv,
                               wo, bo)

    from concourse import bass_utils

    nc = build_program()
    in_maps = make_in_maps(x, router_w, wq, wk, wv, wo)
    res = bass_utils.run_bass_kernel_spmd(nc, in_maps, core_ids=list(range(8)))
    out = np.empty((B, N, C), np.float32)
    for core in range(8):
        b, h = core // 2, core % 2
        out[b, h * NQ:(h + 1) * NQ, :] = res.results[core]["out"]
    return out



# revision 30
# speedup vs baseline: 1.3863x; 1.0340x over previous
"""Trainium2 Bass kernel for MIGAttention (topk token masking + GQA attention).

Shapes (hardcoded): B=4, N=2048, C=1024, H=16 heads, HKV=4 kv-heads, DH=64,
keep-ratio 0.7 -> k = 1433 selected tokens per batch row.

Sharding: 8 cores = (batch b in 0..3) x (query-half h in 0..1).  Each core
receives x[b].T with token columns rolled by h*1024 so that its own query
half always occupies columns 0..1023 -> a single SPMD program for all cores.

Key structure (v2): the top-k mask selects 1433 of 2048 tokens; masked tokens
have zero K/V rows.  The kernel compacts the selected tokens on device
(prefix-sum + indirect-DMA scatter/gather) and runs attention over only
NSEL=1536 gathered key slots (12 chunks instead of 16).  Junk tail slots
(gathered masked tokens) have zero K/V so their logits are exactly 0 and
p=exp(0)=1, standing in one-for-one for masked tokens in the softmax
denominator; the remaining 2048-1536=512 masked keys are a static +512
constant added to the denominator.

QK_FP8: Q/K are quantized to fp8e4 at the projection evict and the logits
matmuls run in DoubleRow perf mode (2x).  R_PAIRS: the last 2*R_PAIRS key
chunks compute r = expm1(y) ~ y + y^2/2 on the Vector engine (one fused
affine_mul op straight from PSUM) in fp8, and their att@V uses fp8 DoubleRow
with the exact Sum(v) rank-1 correction folded into the same PSUM
accumulation.  This keeps all fp8 error on the small residual r (rms ~0.1)
instead of p (~1.0).
"""

import contextlib
import sys

import ml_dtypes
import numpy as np

if "/opt/trn_rl_repo" not in sys.path:
    sys.path.insert(0, "/opt/trn_rl_repo")

import concourse.bass as bass  # noqa: F401
import concourse.bass_isa as bass_isa
import concourse.mybir as mybir
from concourse import bacc
from concourse.tile import TileContext

F32 = mybir.dt.float32
F32R = mybir.dt.float32r
BF16 = mybir.dt.bfloat16
F8E4 = mybir.dt.float8e4
I32 = mybir.dt.int32
I16 = mybir.dt.int16
AF = mybir.ActivationFunctionType
ALU = mybir.AluOpType
DR = mybir.MatmulPerfMode.DoubleRow

B, N, C = 4, 2048, 1024
H, HKV, DH = 16, 4, 64
NQ = N // 2          # queries per core
KSEL = 1433          # max(1, int(N * 0.7))
CC = C // 128        # contraction chunks (8)
KC = N // 128        # dense token chunks (16)
KCG = 12             # gathered key chunks (1536 slots >= KSEL)
NSEL = KCG * 128     # 1536
QT_D = H * DH        # 1024
KV_D = HKV * DH      # 256
N_ROUNDS = 4         # topk threshold refinement rounds (8/128^4 ~ 3e-8 << min topk gap 3.5e-6)
LO0, W0 = -4.0, 8.0  # initial logit search interval (logit std ~0.65)

# ---- tuning flags ----
QK_FP8 = True        # fp8 Q/K + DoubleRow logits matmuls
R_PAIRS = 0          # of the 12 chunks, last 2*R_PAIRS use the DVE r-route
P_KC = KCG - 2 * R_PAIRS
SQK = 32.0           # fp8 quantization scale for q/k
SV = 4.0             # fp8 v scale for the r-route (r is divided by SV)
EXP_SCALE = 1.0 / (np.sqrt(DH) * (SQK * SQK if QK_FP8 else 1.0))
KDT = F8E4 if QK_FP8 else BF16
GSUB, NSUB = 384, NSEL // 384  # SWDGE sub-gather split


def _emit(nc, tc, ctx, io):
    xT, wq, wk, wv, rw, wo, out_d = (
        io["xT"], io["wq"], io["wk"], io["wv"], io["rw"], io["wo"], io["out"])

    # ---------------- long-lived pools ----------------
    const = ctx.enter_context(tc.tile_pool(name="const", bufs=1))
    small = ctx.enter_context(tc.tile_pool(name="small", bufs=1))
    big = ctx.enter_context(tc.tile_pool(name="big", bufs=1))
    dram = ctx.enter_context(tc.tile_pool(name="dram", bufs=1, space="DRAM"))

    # nesting (LIFO): px > pm > {pa, pcs, pg > {pq, pkv}}
    px_ctx = contextlib.ExitStack()   # xT (alive through projections+gather)
    pm_ctx = contextlib.ExitStack()   # m_rep/m_v/compaction sbuf scratch
    pa_ctx = contextlib.ExitStack()   # router/refinement scratch
    pg_ctx = contextlib.ExitStack()   # proj staging + transpose psum
    pq_ctx = contextlib.ExitStack()   # wq
    pkv_ctx = contextlib.ExitStack()  # wk, wv
    px = px_ctx.enter_context(tc.tile_pool(name="px", bufs=1))
    pm = pm_ctx.enter_context(tc.tile_pool(name="pm", bufs=1))
    pa = pa_ctx.enter_context(tc.tile_pool(name="pa", bufs=1))
    psum_r = pa_ctx.enter_context(tc.tile_pool(name="psum_r", bufs=1, space="PSUM"))

    # ---------------- constants ----------------
    ones_row = const.tile([1, 128], F32)
    nc.vector.memset(ones_row, 1.0)
    ones512 = const.tile([1, 512], BF16)
    nc.vector.memset(ones512, 1.0)
    onescol_bf = const.tile([128, 1], BF16)
    nc.vector.memset(onescol_bf, 1.0)
    iota128_i = const.tile([128, 1], I32)
    nc.gpsimd.iota(iota128_i, pattern=[[0, 1]], base=1, channel_multiplier=1)
    iota128 = const.tile([128, 1], F32)
    nc.vector.tensor_copy(iota128, iota128_i)
    sel8 = const.tile([16, CC, 128], F32R)
    nc.sync.dma_start(sel8, io["sel8"].bitcast(F32R))

    # ---------------- DRAM scratch ----------------
    m_dram = dram.tile([N], F32)
    idx16_dram = dram.tile([NSEL], I16)
    k_dram = dram.tile([N, KV_D], BF16)
    v_dram = dram.tile([N, KV_D], BF16)

    # ---------------- router: logits = x @ rw, exact fp32 ----------------
    rw_sb = pa.tile([128, CC], F32)
    for cc in range(CC):
        sl = slice(cc * 128, (cc + 1) * 128)
        nc.sync.dma_start(rw_sb[:, cc:cc + 1], rw[sl, :])
    # Single fp32 x load feeds both the exact-fp32 PE router matmuls (the
    # instruction structure must match the reference summation closely: batch
    # 0's topk threshold gap is 3.5e-6, so any other reduction order flips a
    # near-tie - measured both for f32r and for a DVE-partials variant) and,
    # via engine casts, the bf16 working copy the projections use.
    logits_sb = pa.tile([1, N], F32)
    xr_pool = pa_ctx.enter_context(tc.tile_pool(name="xr_pool", bufs=3))
    xb = px.tile([128, CC, N], BF16)
    rps = [psum_r.tile([1, 512], F32, tag=f"router_ps{g}", name=f"router_ps{g}")
           for g in range(4)]
    for cc in range(CC):
        xr = xr_pool.tile([128, N], F32, tag="xr", name=f"xr{cc}")
        eng = nc.sync if cc % 2 == 0 else nc.scalar
        eng.dma_start(xr, xT[cc * 128:(cc + 1) * 128, :])
        for g in range(4):
            nc.tensor.matmul(
                rps[g], rw_sb[:, cc:cc + 1], xr[:, g * 512:(g + 1) * 512],
                start=(cc == 0), stop=(cc == CC - 1))
        if cc % 2 == 0:
            nc.scalar.copy(xb[:, cc, :], xr)
        else:
            nc.vector.tensor_copy(xb[:, cc, :], xr)
    for g in range(4):
        nc.vector.tensor_copy(logits_sb[:, g * 512:(g + 1) * 512], rps[g])

    # replicate logits across all 128 partitions (K=1 matmul broadcast)
    lrep = pa.tile([128, N], F32)
    for g in range(4):
        ps = psum_r.tile([128, 512], F32, tag="bcast_ps")
        nc.tensor.matmul(ps, ones_row, logits_sb[:, g * 512:(g + 1) * 512],
                         start=True, stop=True)
        nc.vector.tensor_copy(lrep[:, g * 512:(g + 1) * 512], ps)

    # ---------------- topk threshold refinement ----------------
    # invariant: v* (the KSEL-th largest logit) is in (lo, lo + w]
    lo = small.tile([128, 1], F32)
    nc.vector.memset(lo, LO0)
    neg_edges = small.tile([128, 1], F32)
    acc = small.tile([128, 1], F32)
    sel = small.tile([128, 1], F32)
    ssum = small.tile([128, 1], F32)
    sign_scr = pa.tile([128, N], BF16)  # Sign output is never read
    thr_acc = float(2 * KSEL - N)  # acc = #gt - #lt ; acc>=thr <=> #gt>=KSEL
    for r in range(N_ROUNDS):
        wstep = W0 / (128.0 ** (r + 1))
        nc.vector.scalar_tensor_tensor(
            neg_edges, iota128, -wstep, lo, op0=ALU.mult, op1=ALU.subtract)
        nc.scalar.activation(sign_scr, lrep, AF.Sign, bias=neg_edges,
                             scale=1.0, accum_out=acc)
        nc.vector.tensor_single_scalar(sel, acc, thr_acc, op=ALU.is_ge)
        nc.gpsimd.partition_all_reduce(ssum, sel, channels=128,
                                       reduce_op=bass_isa.ReduceOp.add)
        nc.vector.scalar_tensor_tensor(
            lo, ssum, wstep, lo, op0=ALU.mult, op1=ALU.add)

    # m = (logit > lo) * sigmoid(logit)   per token, replicated on partitions
    grep = pa.tile([128, N], F32)
    nc.scalar.activation(grep, lrep, AF.Sigmoid)
    m_rep = pm.tile([128, N], F32)
    nc.vector.scalar_tensor_tensor(
        m_rep, lrep, lo, grep, op0=ALU.is_gt, op1=ALU.mult)

    # token-major copies: m_v[p, i] = m[i*128 + p]
    nc.scalar.dma_start(m_dram, m_rep[0:1, :])
    pa_ctx.close()
    pc = pm
    m_v = pc.tile([128, KC], F32)
    nc.scalar.dma_start(m_v, m_dram.rearrange("(i p) -> p i", p=128))
    if QK_FP8:
        msc_v = pc.tile([128, KC], F32)
        nc.vector.tensor_scalar(msc_v, m_v, SQK, None, op0=ALU.mult)
    else:
        msc_v = m_v

    # ---------------- compaction: selected-token index list ----------------
    # Single-partition row pipeline (pure DVE + one gpsimd scatter; nothing on
    # the PE queue, no DRAM roundtrips): csel[t] = #selected in tokens [0, t],
    # pos[t] = selected ? csel-1 : count + t - csel, clamped to < NSEL.
    m01row = pc.tile([1, N], F32)
    nc.vector.tensor_scalar(m01row, lrep[0:1, :], lo[0:1, :], None,
                            op0=ALU.is_gt)
    csel = pc.tile([1, N], F32)
    nc.vector.tensor_tensor_scan(csel, m01row, m01row, 0.0,
                                 op0=ALU.add, op1=ALU.bypass)
    r_i = pc.tile([1, N], I32)
    nc.gpsimd.iota(r_i, pattern=[[1, N]], base=0, channel_multiplier=0)
    pos = pc.tile([1, N], F32)
    nc.vector.tensor_copy(pos, r_i)           # pos <- t
    nc.vector.tensor_tensor(pos, pos, csel, op=ALU.subtract)
    nc.vector.tensor_scalar(pos, pos, csel[:, N - 1:N], None, op0=ALU.add)
    scr = pc.tile([1, N], F32)
    nc.vector.tensor_scalar(scr, csel, -1.0, None, op0=ALU.add)  # csel-1
    nc.vector.tensor_copy(r_i, m01row)        # int mask
    nc.vector.copy_predicated(pos, r_i, scr)
    # clamp: tokens landing beyond the NSEL gathered slots -> -1 (dropped)
    nc.vector.tensor_single_scalar(scr, pos, float(NSEL), op=ALU.is_lt)
    nc.vector.scalar_tensor_tensor(pos, pos, 1.0, scr,
                                   op0=ALU.add, op1=ALU.mult)
    # single-partition compaction: sc_out[0, pos[t]] = t for pos >= 0
    sc_idx = pc.tile([16, N], I16)
    nc.vector.memset(sc_idx, -1)
    nc.vector.tensor_scalar(sc_idx[0:1, :], pos, -1.0, None, op0=ALU.add)
    sc_data = pc.tile([16, N], I16)
    nc.gpsimd.iota(sc_data, pattern=[[1, N]], base=0, channel_multiplier=0)
    sc_out = pc.tile([16, NSEL], I16)
    nc.gpsimd.local_scatter(sc_out, sc_data, sc_idx, channels=16,
                            num_elems=NSEL, num_idxs=N)
    nc.scalar.dma_start(idx16_dram, sc_out[0:1, :])
    nc.scalar.dma_start(io["dbg_idx"], sc_out[0:1, :])
    nc.scalar.dma_start(io["dbg_m01"], m01row)
    # wrapped gather-index layout: idx j -> [j % 16, j // 16], replicated
    idx16_sb = big.tile([128, NSEL // 16], I16)
    for k in range(8):
        nc.scalar.dma_start(idx16_sb[16 * k:16 * (k + 1), :],
                          idx16_dram.rearrange("(s p) -> p s", p=16))
    stg_pool = pg_ctx.enter_context(tc.tile_pool(name="stg_pool", bufs=3))
    psum1 = pg_ctx.enter_context(tc.tile_pool(name="psum1", bufs=5, space="PSUM"))

    # ---------------- K projection, token-major -> DRAM rows ----------------
    pkv = pkv_ctx.enter_context(tc.tile_pool(name="pkv", bufs=1))
    wk_sb = pkv.tile([128, CC, KV_D], BF16)
    wv_sb = pkv.tile([128, CC, KV_D], BF16)
    for cc in range(CC):
        sl = slice(cc * 128, (cc + 1) * 128)
        nc.sync.dma_start(wk_sb[:, cc, :], wk[sl, :])
        nc.sync.dma_start(wv_sb[:, cc, :], wv[sl, :])
    # ---------------- V projection + gathers ----------------
    for i2 in range(KC // 2):
        vps = psum1.tile([128, 512], F32, tag="proj_ps", name=f"v_ps{i2}")
        for half in range(2):
            i = 2 * i2 + half
            ts_ = slice(i * 128, (i + 1) * 128)
            vp = vps[:, half * KV_D:(half + 1) * KV_D]
            for cc in range(CC):
                nc.tensor.matmul(vp, xb[:, cc, ts_], wv_sb[:, cc, :],
                                 start=(cc == 0), stop=(cc == CC - 1))
        vstage = stg_pool.tile([128, 2, KV_D], BF16, tag="vstage",
                               name=f"vstage{i2}")
        nc.scalar.activation(vstage[:, 0, :], vps[:, 0:KV_D], AF.Identity,
                             scale=m_v[:, 2 * i2:2 * i2 + 1])
        nc.scalar.activation(vstage[:, 1, :], vps[:, KV_D:2 * KV_D],
                             AF.Identity, scale=m_v[:, 2 * i2 + 1:2 * i2 + 2])
        nc.sync.dma_start(
            v_dram[2 * i2 * 128:(2 * i2 + 2) * 128, :]
            .rearrange("(c t) d -> t c d", t=128), vstage)
    v8g = stg_pool.tile([128, KCG, KV_D], BF16, tag="v8g")
    v8_sb = big.tile([128, KCG, HKV, 128], BF16)
    nc.vector.memset(v8_sb.bitcast(I32), 0)
    nc.vector.memset(v8_sb[:, :, :, 64:65], 1.0)
    for g in range(NSUB):
        ic = idx16_sb[:, g * (GSUB // 16):(g + 1) * (GSUB // 16)]
        cs = slice(g * (GSUB // 128), (g + 1) * (GSUB // 128))
        nc.gpsimd.dma_gather(v8g[:, cs, :], v_dram[:, :], ic,
                             GSUB, GSUB, elem_size=KV_D, transpose=False,
                             queue_num=g % 4)
        nc.vector.tensor_copy(
            v8_sb[:, cs, :, 0:64],
            v8g[:, cs, :].rearrange("p c (h e) -> p c h e", e=64))

    # ---------------- K projection, token-major -> DRAM rows ----------------
    for i2 in range(KC // 2):
        kps = psum1.tile([128, 512], F32, tag="proj_ps", name=f"k_ps{i2}")
        for half in range(2):
            i = 2 * i2 + half
            ts_ = slice(i * 128, (i + 1) * 128)
            kp = kps[:, half * KV_D:(half + 1) * KV_D]
            for cc in range(CC):
                nc.tensor.matmul(kp, xb[:, cc, ts_], wk_sb[:, cc, :],
                                 start=(cc == 0), stop=(cc == CC - 1))
        kstage = stg_pool.tile([128, 2, KV_D], BF16, tag="kstage",
                               name=f"kstage{i2}")
        nc.scalar.activation(kstage[:, 0, :], kps[:, 0:KV_D], AF.Identity,
                             scale=msc_v[:, 2 * i2:2 * i2 + 1])
        nc.scalar.activation(kstage[:, 1, :], kps[:, KV_D:2 * KV_D],
                             AF.Identity,
                             scale=msc_v[:, 2 * i2 + 1:2 * i2 + 2])
        nc.sync.dma_start(
            k_dram[2 * i2 * 128:(2 * i2 + 2) * 128, :]
            .rearrange("(c t) d -> t c d", t=128), kstage)

    # ---------------- K gathers (SWDGE), pipelined into kt_z ----------------
    # dma_gather transpose mode: out[d % 128, d // 128, slot] = k[idx[slot], d]
    # -> directly the [dims-on-partitions, token-free] layout the logits
    # matmuls need.  kt_z: two parity copies with the other 64-partition half
    # zeroed, so the logits matmul contracts a full K=128 (partner q rows hit
    # zeros) and the PE HAM activity monitor stays warm.  QK_FP8 adds a
    # zeroed second DoubleRow subtile.  Sub-gathers stay well under the SWDGE
    # descriptor-ring capacity (16KB carveout = 1024 descriptors).
    kt_all = stg_pool.tile([128, NSUB, 2, GSUB], BF16, tag="kt_all")
    if QK_FP8:
        kt_z = big.tile([128, 2, 2, 2, NSEL], F8E4)   # [p, par, sub, j, t]
        nc.vector.memset(kt_z[64:128, 0, :, :, :].bitcast(I32), 0)
        nc.vector.memset(kt_z[0:64, 1, :, :, :].bitcast(I32), 0)
        nc.vector.memset(kt_z[:, :, 1, :, :].bitcast(I32), 0)
    else:
        kt_z = big.tile([128, 2, 2, NSEL], BF16)      # [p, par, j, t]
        nc.vector.memset(kt_z[64:128, 0, :, :].bitcast(I32), 0)
        nc.vector.memset(kt_z[0:64, 1, :, :].bitcast(I32), 0)
    for g in range(NSUB):
        ic = idx16_sb[:, g * (GSUB // 16):(g + 1) * (GSUB // 16)]
        nc.gpsimd.dma_gather(kt_all[:, g, :, :], k_dram[:, :], ic,
                             GSUB, GSUB, elem_size=KV_D, transpose=True,
                             queue_num=g % 4)
        gs = slice(g * GSUB, (g + 1) * GSUB)
        for j in range(2):
            if QK_FP8:
                nc.vector.tensor_copy(kt_z[0:64, 0, 0, j, gs],
                                      kt_all[0:64, g, j, :])
                nc.vector.tensor_copy(kt_z[64:128, 1, 0, j, gs],
                                      kt_all[64:128, g, j, :])
            else:
                nc.vector.tensor_copy(kt_z[0:64, 0, j, gs],
                                      kt_all[0:64, g, j, :])
                nc.vector.tensor_copy(kt_z[64:128, 1, j, gs],
                                      kt_all[64:128, g, j, :])

    pkv_ctx.close()
    # ---------------- QT projection (overlaps the gather chain) ------------
    # Slot layout is permuted so each q-head lands on the same partition range
    # as its GQA kv-head in KT: head h -> slot (h%4)+4*(h//8), partition base
    # ((h//4)%2)*64.  Slot j holds heads (ha, ha+4), ha = j if j<4 else j+4.
    pq = pq_ctx.enter_context(tc.tile_pool(name="pq", bufs=1))
    wq_sb = pq.tile([128, CC, QT_D], BF16)
    for cc in range(CC):
        nc.sync.dma_start(wq_sb[:, cc, :], wq[cc * 128:(cc + 1) * 128, :])
    if QK_FP8:
        qt_sb = big.tile([128, H // 2, 2, NQ], F8E4)
        nc.vector.memset(qt_sb[:, :, 1, :].bitcast(I32), 0)
    else:
        qt_sb = big.tile([128, H // 2, NQ], BF16)
    for j in range(H // 2):
        for g in range(NQ // 512):
            ps = psum1.tile([128, 512], F32, tag="proj_ps",
                            name=f"q_ps{j}_{g}")
            qs = slice(g * 512, (g + 1) * 512)
            for cc in range(CC):
                nc.tensor.matmul(
                    ps, wq_sb[:, cc, j * 128:(j + 1) * 128],
                    xb[:, cc, qs],
                    start=(cc == 0), stop=(cc == CC - 1))
            if QK_FP8:
                nc.vector.scalar_tensor_tensor(
                    qt_sb[:, j, 0, qs], ps, SQK, m_rep[:, qs],
                    op0=ALU.mult, op1=ALU.mult)
            else:
                nc.vector.tensor_tensor(qt_sb[:, j, qs], ps, m_rep[:, qs],
                                        op=ALU.mult)
    pq_ctx.close()

    # r-route prep: fp8 V copies + exact Sum(v) over the r-chunks
    if R_PAIRS:
        psum_t = pg_ctx.enter_context(
            tc.tile_pool(name="psum_t", bufs=1, space="PSUM"))
        v8f = big.tile([128, R_PAIRS, 2, HKV, 128], F8E4)
        vsum_sb = big.tile([1, HKV, 128], BF16)
        vs_ps = psum_t.tile([1, 512], F32, tag="vs", name="vs_ps")
        for pi in range(R_PAIRS):
            for s in range(2):
                rc = P_KC + 2 * pi + s
                nc.vector.tensor_scalar(
                    v8f[:, pi, s, :, :], v8_sb[:, rc, :, :], SV, None,
                    op0=ALU.mult)
                nc.tensor.matmul(vs_ps, onescol_bf, v8_sb[:, rc, :, :],
                                 start=(pi == 0 and s == 0),
                                 stop=(pi == R_PAIRS - 1 and s == 1))
        nc.vector.tensor_copy(vsum_sb, vs_ps)
    pg_ctx.close()
    pm_ctx.close()
    px_ctx.close()  # free xT + phase-1 PSUM

    # ---------------- phase 2: attention ----------------
    ph2_ctx = contextlib.ExitStack()
    ph2 = ph2_ctx.enter_context(tc.tile_pool(name="ph2", bufs=1))
    wo_sb = ph2.tile([128, CC, C], F32R)
    for cc in range(CC):
        nc.sync.dma_start(wo_sb[:, cc, :],
                          wo[cc * 128:(cc + 1) * 128, :].bitcast(F32R))

    patt_ctx = contextlib.ExitStack()
    scr_pool = patt_ctx.enter_context(tc.tile_pool(name="scr_pool", bufs=2))
    p_pool = patt_ctx.enter_context(tc.tile_pool(name="p_pool", bufs=2))
    lg_pool = patt_ctx.enter_context(
        tc.tile_pool(name="lg_pool", bufs=2, space="PSUM"))
    att_pool = patt_ctx.enter_context(
        tc.tile_pool(name="att_pool", bufs=1, space="PSUM"))
    oT_sb = ph2.tile([128, CC, NQ], F32R)
    denom_sb = ph2.tile([16, NQ], F32)

    def lg_matmuls(lg, pair, j, kc):
        """logits for both heads of the pair into lg[0], lg[1]."""
        for m in range(2):
            jq = (pair[m] % 4) + 4 * (pair[m] // 8)
            for g in range(NQ // 512):
                gs = slice(g * 512, (g + 1) * 512)
                if QK_FP8:
                    nc.tensor.matmul(
                        lg[m][:, gs],
                        kt_z[:, m, :, j, kc * 128:(kc + 1) * 128],
                        qt_sb[:, jq, :, gs],
                        start=True, stop=True, perf_mode=DR,
                        skip_group_check=True)
                else:
                    nc.tensor.matmul(
                        lg[m][:, gs],
                        kt_z[:, m, j, kc * 128:(kc + 1) * 128],
                        qt_sb[:, jq, gs],
                        start=True, stop=True)

    pair_heads = [(ha, ha + 4) for ha in (0, 1, 2, 3, 8, 9, 10, 11)]
    for hp, pair in enumerate(pair_heads):
        j = pair[0] // 8
        att_ps = [att_pool.tile([128, NQ], F32, tag=f"att{m}", name=f"att{hp}_{m}")
                  for m in range(2)]
        pend = []  # pipelined attv matmuls: emitted one chunk behind
        for quarter in range(P_KC // 2):
            p_t = p_pool.tile([128, 2, N], BF16, tag="p_t",
                              name=f"p_{hp}_{quarter}")
            for kci in range(2):
                kc = quarter * 2 + kci
                lg = [lg_pool.tile([128, NQ], F32, tag="lg",
                                   name=f"lg{hp}_{kc}_{m2}") for m2 in range(2)]
                lg_matmuls(lg, pair, j, kc)
                for m in range(2):
                    nc.scalar.activation(
                        p_t[:, kci, m * NQ:(m + 1) * NQ], lg[m], AF.Exp,
                        scale=EXP_SCALE)
                for f in pend:
                    f()
                pend = []

                def attv(p_t=p_t, kci=kci, kc=kc):
                    for m in range(2):
                        hk = pair[m] // 4
                        for g in range(NQ // 512):
                            nc.tensor.matmul(
                                att_ps[m][:, g * 512:(g + 1) * 512],
                                v8_sb[:, kc, hk, :],
                                p_t[:, kci,
                                    m * NQ + g * 512:m * NQ + (g + 1) * 512],
                                start=(kc == 0),
                                stop=(kc == KCG - 1 and not R_PAIRS),
                                skip_group_check=True)

                pend.append(attv)
        for pi in range(R_PAIRS):
            r8 = p_pool.tile([128, 2, N], F8E4, tag="r8", name=f"r8_{hp}_{pi}")
            racc = scr_pool.tile([128, 1], F32, tag="racc", name=f"racc{hp}{pi}")
            for s in range(2):
                kc = P_KC + 2 * pi + s
                lg = [lg_pool.tile([128, NQ], F32, tag="lg",
                                   name=f"lg{hp}_{kc}_{m2}") for m2 in range(2)]
                lg_matmuls(lg, pair, j, kc)
                # r = (z + z^2/2)/SV, z = y*c: t = 1 + z/2 (DVE, psum->
                # sbuf), then (y*(c/SV))*t (one PSUM operand only)
                for m in range(2):
                    tq = scr_pool.tile([128, NQ], BF16, tag="tq",
                                       name=f"tq{hp}_{pi}_{s}_{m}")
                    nc.vector.tensor_scalar(
                        tq, lg[m], float(EXP_SCALE / 2.0), 1.0,
                        op0=ALU.mult, op1=ALU.add)
                    nc.vector.affine_mul_reduce(
                        r8[:, s, m * NQ:(m + 1) * NQ], racc, lg[m], tq,
                        scale=float(EXP_SCALE / SV), bias=0.0)
                for f in pend:
                    f()
                pend = []

            def attv_r(r8=r8, pi=pi):
                for m in range(2):
                    hk = pair[m] // 4
                    for g in range(NQ // 512):
                        nc.tensor.matmul(
                            att_ps[m][:, g * 512:(g + 1) * 512],
                            v8f[:, pi, :, hk, :],
                            r8[:, :, m * NQ + g * 512:m * NQ + (g + 1) * 512],
                            start=False, stop=False, perf_mode=DR,
                            skip_group_check=True)

            pend.append(attv_r)
        for f in pend:
            f()
        if R_PAIRS:
            # rank-1 correction: + Sum_{r-chunks} v (incl. slot count in the
            # ones column) to every query column
            for m in range(2):
                hk = pair[m] // 4
                for g in range(NQ // 512):
                    nc.tensor.matmul(
                        att_ps[m][:, g * 512:(g + 1) * 512],
                        vsum_sb[0:1, hk, :], ones512,
                        start=False, stop=True, skip_group_check=True)
        # fast evict: copy [65, NQ] psum -> sbuf scratch, stash denom row,
        # numerator into oT unscaled; 1/denom once after all pairs.
        for m in range(2):
            h = pair[m]
            scr65 = scr_pool.tile([65, NQ], F32R, tag="scr65",
                                  name=f"scr65_{hp}_{m}")
            nc.vector.tensor_copy(scr65, att_ps[m][0:65, :])
            nc.sync.dma_start(denom_sb[h:h + 1, :],
                              scr65[64:65, :].bitcast(F32))
            if h % 2 == 0:
                nc.vector.tensor_copy(oT_sb[0:64, h // 2, :], scr65[0:64, :])
            else:
                # partition shift 0 -> 64 must go through DMA
                nc.sync.dma_start(oT_sb[64:128, h // 2, :], scr65[0:64, :])

    # denominator: + (N - NSEL) for the never-gathered masked keys, then
    # one batched reciprocal and a per-d-chunk broadcast multiply.
    nc.vector.tensor_scalar(denom_sb, denom_sb, float(N - NSEL), None,
                            op0=ALU.add)
    rec16 = ph2.tile([16, NQ], F32R)
    rec16_f = ph2.tile([16, NQ], F32)
    with nc.allow_low_precision(reason="2e-5 rel err << output tolerance"):
        nc.vector.reciprocal_approx_fast(out=rec16_f, in_=denom_sb)
    nc.vector.tensor_copy(rec16, rec16_f)
    for dd in range(CC):
        for g in range(NQ // 512):
            bps = lg_pool.tile([128, 512], F32, tag="lg", name=f"bps{dd}_{g}")
            nc.tensor.matmul(
                bps, sel8[:, dd, :], rec16[:, g * 512:(g + 1) * 512],
                start=True, stop=True)
            sl = slice(g * 512, (g + 1) * 512)
            nc.vector.tensor_tensor(
                oT_sb[:, dd, sl], oT_sb[:, dd, sl], bps, op=ALU.mult)
    patt_ctx.close()
    # ---------------- phase 3: output projection ----------------
    ph3_ctx = contextlib.ExitStack()
    psum3 = ph3_ctx.enter_context(tc.tile_pool(name="psum3", bufs=4, space="PSUM"))
    out_pool = ph3_ctx.enter_context(tc.tile_pool(name="out_pool", bufs=2))
    for tt in range(NQ // 128):
        out_sb = out_pool.tile([128, C], F32, tag="out_sb", name=f"out_sb{tt}")
        for og in range(C // 512):
            ps = psum3.tile([128, 512], F32, tag="out_ps", name=f"out_ps{tt}_{og}")
            for dd in range(CC):
                nc.tensor.matmul(
                    ps, oT_sb[:, dd, tt * 128:(tt + 1) * 128],
                    wo_sb[:, dd, og * 512:(og + 1) * 512],
                    start=(dd == 0), stop=(dd == CC - 1))
            nc.scalar.copy(out_sb[:, og * 512:(og + 1) * 512], ps)
        nc.sync.dma_start(out_d[tt * 128:(tt + 1) * 128, :], out_sb)
    ph3_ctx.close()
    ph2_ctx.close()


_NC = None


def build_program():
    global _NC
    if _NC is not None:
        return _NC
    from contextlib import ExitStack

    nc = bacc.Bacc("TRN2", target_bir_lowering=False, debug=False, num_devices=8,
                   num_swdge_queues=4)
    io = {
        "xT": nc.dram_tensor("xT", (C, N), F32, kind="ExternalInput").ap(),
        "wq": nc.dram_tensor("wq", (C, QT_D), BF16, kind="ExternalInput").ap(),
        "wk": nc.dram_tensor("wk", (C, KV_D), BF16, kind="ExternalInput").ap(),
        "wv": nc.dram_tensor("wv", (C, KV_D), BF16, kind="ExternalInput").ap(),
        "rw": nc.dram_tensor("rw", (C, 1), F32, kind="ExternalInput").ap(),
        "wo": nc.dram_tensor("wo", (C, C), F32, kind="ExternalInput").ap(),
        "sel8": nc.dram_tensor("sel8", (16, CC, 128), F32,
                               kind="ExternalInput").ap(),
        "out": nc.dram_tensor("out", (NQ, C), F32, kind="ExternalOutput").ap(),
        "dbg_idx": nc.dram_tensor("dbg_idx", (NSEL,), I16,
                                  kind="ExternalOutput").ap(),
        "dbg_m01": nc.dram_tensor("dbg_m01", (N,), F32,
                                  kind="ExternalOutput").ap(),
    }
    with TileContext(nc) as tc:
        with ExitStack() as ctx:
            _emit(nc, tc, ctx, io)
    nc.compile()
    _NC = nc
    return nc


def _permute_wq(wq):
    """Column-permute wq so QT slot j's 128 cols = heads (ha, ha+4) contig."""
    wq = np.asarray(wq, np.float32).reshape(C, H, DH)
    order = []
    for j in range(H // 2):
        ha = j if j < 4 else j + 4
        order += [ha, ha + 4]
    return np.ascontiguousarray(wq[:, order, :].reshape(C, H * DH))


def make_in_maps(x, router_w, wq, wk, wv, wo):
    wq = _permute_wq(wq)
    in_maps = []
    for core in range(8):
        b, h = core // 2, core % 2
        xT_core = np.ascontiguousarray(
            np.roll(np.asarray(x[b], np.float32).T, -h * NQ, axis=1))
        sel8 = np.zeros((16, CC, 128), np.float32)
        for dd in range(CC):
            for p in range(128):
                sel8[2 * dd + p // 64, dd, p] = 1.0
        in_maps.append({
            "xT": xT_core,
            "sel8": sel8,
            "wq": np.ascontiguousarray(np.asarray(wq, np.float32),
                                       dtype=ml_dtypes.bfloat16),
            "wk": np.ascontiguousarray(np.asarray(wk, np.float32),
                                       dtype=ml_dtypes.bfloat16),
            "wv": np.ascontiguousarray(np.asarray(wv, np.float32),
                                       dtype=ml_dtypes.bfloat16),
            "rw": np.ascontiguousarray(router_w, dtype=np.float32),
            "wo": np.ascontiguousarray(wo, dtype=np.float32),
        })
    return in_maps


def _numpy_fallback(x, router_w, router_b, wq, bq, wk, bk, wv, bv, wo, bo):
    x = np.asarray(x, np.float32)
    gate = 1.0 / (1.0 + np.exp(-(x @ router_w + router_b)))
    xg = x * gate
    scores = gate[..., 0]
    idx = np.argsort(-scores, axis=-1, kind="stable")[:, :KSEL]
    mask = np.zeros((x.shape[0], x.shape[1]), np.float32)
    np.put_along_axis(mask, idx, 1.0, axis=1)
    xg = xg * mask[..., None]
    q = (xg @ wq + bq).reshape(B, N, H, DH)
    kk = np.repeat((xg @ wk + bk).reshape(B, N, HKV, DH), H // HKV, axis=2)
    v = np.repeat((xg @ wv + bv).reshape(B, N, HKV, DH), H // HKV, axis=2)
    att = np.einsum("bqhd,bkhd->bhqk", q, kk) / np.float32(np.sqrt(DH))
    att = att - att.max(-1, keepdims=True)
    att = np.exp(att)
    att = att / att.sum(-1, keepdims=True)
    o = np.einsum("bhqk,bkhd->bqhd", att, v).reshape(B, N, C)
    return (o @ wo + bo).astype(np.float32)


def kernel(x, router_w, router_b, wq, bq, wk, bk, wv, bv, wo, bo):
    x = np.asarray(x)
    biases = [router_b, bq, bk, bv, bo]
    if any(float(np.abs(np.asarray(t)).max()) != 0.0 for t in biases):
        # The device program folds away the (identically zero) biases; fall
        # back to an exact host implementation if that assumption breaks.
        return _numpy_fallback(x, router_w, router_b, wq, bq, wk, bk, wv, bv,
                               wo, bo)

    from concourse import bass_utils

    nc = build_program()
    in_maps = make_in_maps(x, router_w, wq, wk, wv, wo)
    res = bass_utils.run_bass_kernel_spmd(nc, in_maps, core_ids=list(range(8)))
    out = np.empty((B, N, C), np.float32)
    for core in range(8):
        b, h = core // 2, core % 2
        out[b, h * NQ:(h + 1) * NQ, :] = res.results[core]["out"]
    return out

